# revision 13
# baseline (speedup 1.0000x reference)
"""Multi-head attention (B=2, S=2048, H=1024, 16 heads x 64) on 8 trn2 cores.

Sharding: core c handles batch b=c//4 and the 4 heads [4*(c%4) .. 4*(c%4)+3]
(tensor-parallel over the hd=256 column slice of Wq/Wk/Wv and the matching
row slice of Wo).  Each core computes a rank-256 partial of the output
projection for its batch; the host sums the 4 partials per batch and adds bo.

Device kernel (per core, bf16 matmuls with fp32 PSUM accumulate):
  QT[hd,s] = Wq_c^T X_b^T   (lhsT=Wq nat. layout, rhs=X^T prepped on host)
  KT[hd,s] similarly, stored zero-padded to K=128 per head slot (K=64
  matmuls stream ~1.5x slower per column on the PE), V[s,hd] as [ones|V_h].
  Per head pair (2 heads packed in the 128-partition dim):
    ST[k,q]  = [KT_h;0]^T QT_pair    (full K=128, zero rows null the other
                                      head's rows of the shared qt rhs)
    PT       = exp(SCALE*ST + maskbias[k])   (ScalarE, mask folded into bias)
    [d;OT]   = [ones64|V_h]^T PT     (M=128: rows 0:64 d replicated, 64:128 OT
                                      -> no separate denominator matmul)
    OT_norm  = OT * (1/d)            (reciprocal_approx_fast at base
                                      partition 0 + cross-partition DVE mults)
  Y_partial[s,H] = OT_norm^T Wo_c   (fp16 output tiles, host sums partials)

Scheduling: one flat software pipeline; ScalarE's exp paces the attention
stream while the PE runs PV two iterations behind it (avoids the psum
handoff + fresh-SBUF-read latency on pt).  Projection groups (QT/KT/V) and
output tiles drain from a deadline/credit-paced backlog between iterations
so the PE queue stays fed for the whole run.  PSUM pool creation order is
load-bearing (bank-group conflicts between the exp read stream and
concurrent matmul writes; see the pool comments).
"""
import sys

sys.path.insert(0, "/opt/trn_rl_repo")

import numpy as np
import ml_dtypes
from contextlib import ExitStack

B, S, H = 2, 2048, 1024
NH, HD = 16, 64
SCALE = 1.0 / float(np.sqrt(HD))
HPC = 4          # heads per core
HDC = HPC * HD   # 256 per-core head-dim slice
P = 128
KO = H // P      # 8 contraction tiles for the projections
ST_TILES = S // P    # 16
NQ = S // 512        # 4 q-chunks of 512
M2 = HDC // P        # 2 partition-tiles of the per-core head dim

_BUILT = {}


def _build(dt_name="bfloat16"):
    import concourse.bacc as bacc
    import concourse.mybir as mybir
    import concourse.tile as tile

    DT = getattr(mybir.dt, dt_name)
    F32 = mybir.dt.float32

    nc = bacc.Bacc("TRN2", target_bir_lowering=False, debug=False)

    # all inputs pre-rearranged on host so DMAs are per-partition contiguous
    xt_d = nc.dram_tensor("xt", [NQ, P, KO, 512], DT, kind="ExternalInput").ap()
    wq_d = nc.dram_tensor("wq", [P, KO, HDC], DT, kind="ExternalInput").ap()
    wk_d = nc.dram_tensor("wk", [P, KO, HDC], DT, kind="ExternalInput").ap()
    wv_d = nc.dram_tensor("wv", [P, KO, HDC], DT, kind="ExternalInput").ap()
    wo_d = nc.dram_tensor("wo", [P, M2, H], DT, kind="ExternalInput").ap()
    small_d = nc.dram_tensor("small", [P, M2 + M2 + HDC + ST_TILES], F32,
                             kind="ExternalInput").ap()
    F16 = mybir.dt.float16
    y_d = nc.dram_tensor("y", [S, H], F16, kind="ExternalOutput").ap()

    with tile.TileContext(nc) as tc, ExitStack() as ctx:
        consts = ctx.enter_context(tc.tile_pool(name="consts", bufs=1))
        y_pool = ctx.enter_context(tc.tile_pool(name="ysb", bufs=4))
        qkv = ctx.enter_context(tc.tile_pool(name="qkv", bufs=1))
        pt_pool = ctx.enter_context(tc.tile_pool(name="pt", bufs=5))
        sm_pool = ctx.enter_context(tc.tile_pool(name="sm", bufs=4))
        # PSUM bank-group placement matters: this order puts proj in {0,1},
        # the two ST buffers in {2,3}/{4,5} (so the exp read alternates
        # 4-bank groups) and the PV pair in {6,7} -- the measured conflict
        # tax is ~16us; other orders measured up to +40us worse.
        ps_proj = ctx.enter_context(tc.tile_pool(name="ps_proj", bufs=2, space="PSUM"))
        ps_st = ctx.enter_context(tc.tile_pool(name="ps_st", bufs=2, space="PSUM"))
        ps_ot = ctx.enter_context(tc.tile_pool(name="ps_ot", bufs=1, space="PSUM"))

        # ---- engine warmup during the input-DMA window ----
        # ~8 dummy matmuls trip the PE HAM clock-gate (3.4us busy window ->
        # 2.4GHz) and a dummy exp preloads the ScalarE Exp table, both before
        # any real work depends on them.
        ones_sb = consts.tile([P, 64], DT)
        nc.vector.memset(ones_sb[:], 1.0)
        warm_sb = consts.tile([P, 512], DT)
        nc.vector.memset(warm_sb[:], 1.0)
        warm_out = consts.tile([P, 64], DT)
        warm_ps = ps_proj.tile([P, 512], F32, tag="ps", name="warm_ps")
        for _ in range(8):
            nc.tensor.matmul(warm_ps[:], lhsT=warm_sb[:, 0:128], rhs=warm_sb[:],
                             start=True, stop=True)
        nc.scalar.activation(warm_out[:], ones_sb[:],
                             mybir.ActivationFunctionType.Exp,
                             bias=0.0, scale=1.0)

        # ---- input DMAs, strictly criticality-ordered ----
        # The first exp is gated on wk + xt chunk0 -> kt0/qt0 groups -> st0.
        # Those two transfers get the two fastest slots with nothing else
        # competing for HBM: scalar queue runs wk,wq,small,wo; sync queue
        # runs chunk0,wv,chunk1..3.  gpsimd stays empty so it never steals
        # bandwidth from the critical pair (measured: chunk0 landing at
        # ~21us with 3-way competition vs ~13us without).
        wk_sb = consts.tile([P, KO, HDC], DT)
        nc.scalar.dma_start(wk_sb[:], wk_d)
        xt_sb = consts.tile([P, KO, S], DT)
        nc.sync.dma_start(xt_sb[:, :, 0:512], xt_d[0])
        wq_sb = consts.tile([P, KO, HDC], DT)
        nc.scalar.dma_start(wq_sb[:], wq_d)
        wv_sb = consts.tile([P, KO, HDC], DT)
        nc.sync.dma_start(wv_sb[:], wv_d)

        # small per-partition constants ride one packed DMA (dma_start issue
        # cost on the sequencers is ~0.6us each)
        small_sb = consts.tile([P, M2 + M2 + HDC + ST_TILES], F32)
        nc.scalar.dma_start(small_sb[:], small_d)
        bqt_sb = small_sb[:, 0:M2]
        bkt_sb = small_sb[:, M2:2 * M2]
        bvr_sb = small_sb[:, 2 * M2:2 * M2 + HDC]
        mb_sb = small_sb[:, 2 * M2 + HDC:]

        nc.sync.dma_start(xt_sb[:, :, 512:1024], xt_d[1])
        nc.sync.dma_start(xt_sb[:, :, 1024:1536], xt_d[2])
        nc.sync.dma_start(xt_sb[:, :, 1536:2048], xt_d[3])
        wo_sb = consts.tile([P, M2, H], DT)
        nc.scalar.dma_start(wo_sb[:], wo_d)

        qt_sb = qkv.tile([P, M2, S], DT)
        # KT natural layout [hd-pair 128, m, S]: rows 0:64 head 2m, rows
        # 64:128 head 2m+1.  The ST matmuls run as two CONCURRENT K=64
        # row-tiles (tile_position (0,0)/(64,0) auto-derived from the
        # base partitions) -- both heads' score tiles stream in ~512
        # cycles instead of 1024 with the old zero-padded K=128 form.
        kt_sb = qkv.tile([P, M2, S], DT)
        # per head h: cols 0:64 = ones, cols 64:128 = V_h (so the PV lhsT
        # [ones|V_h] computes the softmax denominator replicated across rows
        # 0:64 -- base partition 0, where reciprocal_approx_fast works --
        # and OT in rows 64:128, in the same matmul)
        v_sb = qkv.tile([P, ST_TILES, HPC, P], DT)
        nc.vector.memset(v_sb[:, :, :, 0:HD], 1.0)
        ot_sb = qkv.tile([P, M2, S], DT)

        # ---- projection group emitters ----
        # A spec describes one 8-matmul accumulation group; emitting two specs
        # interleaved lets each group's LDWEIGHTS prefetch under the other
        # group's matmul streaming.
        def _qk_ops(args):
            w_sb, b_sb, out_sb, m, q = args
            qs = slice(q * 512, (q + 1) * 512)
            ps = ps_proj.tile([P, 512], F32, tag="ps", name="ps_qk")

            def mm(ko, start, stop):
                nc.tensor.matmul(
                    ps[:],
                    lhsT=w_sb[:, ko, m * P:(m + 1) * P],
                    rhs=xt_sb[:, ko, qs],
                    start=start, stop=stop,
                )

            def finish():
                nc.vector.tensor_add(
                    out_sb[:, m, qs], ps[:],
                    b_sb[:, m:m + 1].to_broadcast((P, 512)),
                )

            return mm, finish

        def _v_ops(args):
            (st,) = args
            ps_full = ps_proj.tile([P, 512], F32, tag="ps", name="ps_v")
            ps = ps_full[:, :HDC]

            def mm(ko, start, stop):
                nc.tensor.matmul(
                    ps[:],
                    lhsT=xt_sb[:, ko, st * P:(st + 1) * P],
                    rhs=wv_sb[:, ko, :],
                    start=start, stop=stop,
                )

            def finish():
                for h in range(HPC):
                    hs = slice(h * HD, (h + 1) * HD)
                    nc.vector.tensor_add(v_sb[:, st, h, HD:P], ps[:, hs],
                                         bvr_sb[:, hs])

            return mm, finish

        def emit_group(kind, args):
            mm, finish = (_qk_ops if kind == "qk" else _v_ops)(args)
            for ko in range(KO):
                mm(ko, ko == 0, ko == KO - 1)
            finish()

        def emit_proj(specs):
            for kind, args in specs:
                emit_group(kind, args)

        def emit_y_tile(st, n):
            ss = slice(st * P, (st + 1) * P)
            ns = slice(n * 512, (n + 1) * 512)
            yp = ps_proj.tile([P, 512], F32, tag="ps", name="yp")
            for m in range(M2):
                nc.tensor.matmul(
                    yp[:],
                    lhsT=ot_sb[:, m, ss], rhs=wo_sb[:, m, ns],
                    start=(m == 0), stop=(m == M2 - 1),
                )
            y_sb = y_pool.tile([P, 512], F16, name="y_sb")
            # psum->sbuf copies ride the DVE during the exp stream (a ScalarE
            # copy steals ~0.7us from the exp pacer); only the final q-group
            # -- after the last exp -- alternates onto the idle ScalarE so
            # the tail copy chain runs on two engines.
            if st >= 12 and (st * 2 + n) % 2 == 1:
                nc.scalar.activation(y_sb[:], yp[:],
                                     mybir.ActivationFunctionType.Copy)
            else:
                nc.vector.tensor_copy(y_sb[:], yp[:])
            if (st * 2 + n) % 2 == 0:
                nc.sync.dma_start(y_d[ss, ns], y_sb[:])
            else:
                nc.gpsimd.dma_start(y_d[ss, ns], y_sb[:])

        # ---- backlog of work drained through the pipeline ----
        # items: (deadline_iter, seq, kind, args, pe_ns); kept sorted by
        # deadline.  Groups are atomic (psum accumulation must not
        # interleave with another group sharing the 2-buf proj pool).
        backlog = []
        _seq_no = [0]
        _COST = {"qk": 2240, "v": 920, "y": 490}

        def add(deadline, kind, *args):
            backlog.append((deadline, _seq_no[0], kind, args, _COST[kind]))
            _seq_no[0] += 1

        def run_item(kind, args):
            if kind == "y":
                emit_y_tile(*args)
            else:
                emit_group(kind, args)

        # discretionary drain pacing: PE has ~200ns/iter spare beside the
        # exp-paced ST/PV stream, but total backlog exceeds total spare, so
        # feed steadily (~600ns/iter) to keep the PE queue non-empty for the
        # whole attention phase instead of exhausting the backlog early and
        # starving the PE late.
        _credit = [0.0]

        def drain(i):
            backlog.sort(key=lambda t: (t[0], t[1]))
            _credit[0] += 620.0
            while backlog:
                dl, _, kind, args, cost = backlog[0]
                if dl <= i + 2 or _credit[0] >= cost:
                    backlog.pop(0)
                    run_item(kind, args)
                    _credit[0] = max(_credit[0] - cost, -2240.0)
                else:
                    break

        # attention group order: q-outer spreads Y work across the pipeline
        seq = [(q, m, kt) for q in range(NQ) for m in range(M2)
               for kt in range(ST_TILES)]
        giter = {}  # (q, m) -> start iter
        for i, (q, m, kt) in enumerate(seq):
            if kt == 0:
                giter[(q, m)] = i

        # prefix: just enough for the pipeline to start (v0/v1 are emitted
        # right after the first ST matmuls below, so the first exp isn't
        # queued behind them)

        # backlog deadlines
        for m in range(M2):
            for j in range(NQ):
                if (m, j) != (0, 0):
                    # KT chunk j needed by kt=4j of every group of this m
                    add(giter[(0, m)] + 4 * j if (m, j) != (0, 0) else 0,
                        "qk", wk_sb, bkt_sb, kt_sb, m, j)
                if (m, j) != (0, 0):
                    add(giter[(j, m)], "qk", wq_sb, bqt_sb, qt_sb, m, j)
        for st in range(2, ST_TILES):
            add(st, "v", st)

        # ---- flat attention pipeline ----
        def st_mms(q, m, kt):
            ks = slice(kt * P, (kt + 1) * P)
            qs = slice(q * 512, (q + 1) * 512)
            stp = ps_st.tile([P, 1024], F32, name="stp", tag="stp")
            # two concurrent K=64 row-tiles (auto tile_position (0,0) and
            # (64,0)): head A scores to cols 0:512, head B to 512:1024
            nc.tensor.matmul(
                stp[:, 0:512],
                lhsT=kt_sb[0:64, m, ks], rhs=qt_sb[0:64, m, qs],
                start=True, stop=True,
            )
            nc.tensor.matmul(
                stp[:, 512:1024],
                lhsT=kt_sb[64:128, m, ks], rhs=qt_sb[64:128, m, qs],
                start=True, stop=True,
            )
            return stp

        # PV runs two iterations BEHIND the exp pacer: at iter i the PE
        # consumes pt_{i-2}.  With lag 1 the PE (faster per-iter than
        # ScalarE once the backlog drains) catches up and stalls ~160ns per
        # iteration waiting on exp_{i-1} completing plus the fresh SBUF
        # write->read turnaround on pt; lag 2 gives a full exp period of
        # slack.
        _ot_ps = [None]

        def emit_pv(idx, i_emit, pt):
            q, m, kt = idx
            qs = slice(q * 512, (q + 1) * 512)
            hA, hB = 2 * m, 2 * m + 1
            if kt == 0:
                _ot_ps[0] = ps_ot.tile([P, 1024], F32, name="ot_ps")
            ot_ps = _ot_ps[0]
            # [d;OT] accumulation: lhsT [ones|V_h] -> rows 0:64 denominator
            # replicated, rows 64:128 OT_h (head A cols 0:512 of the pair
            # psum, head B cols 512:1024)
            nc.tensor.matmul(
                ot_ps[:, 0:512],
                lhsT=v_sb[:, kt, hA, :], rhs=pt[:, 0:512],
                start=(kt == 0), stop=(kt == ST_TILES - 1),
            )
            nc.tensor.matmul(
                ot_ps[:, 512:1024],
                lhsT=v_sb[:, kt, hB, :], rhs=pt[:, 512:1024],
                start=(kt == 0), stop=(kt == ST_TILES - 1),
            )
            if kt == ST_TILES - 1:
                rec = sm_pool.tile([P, 1024], F32, tag="rec", name="rec")
                nc.vector.reciprocal_approx_fast(rec[0:64, :], ot_ps[0:64, :])
                nc.vector.tensor_mul(ot_sb[0:64, m, qs],
                                     ot_ps[64:128, 0:512],
                                     rec[0:64, 0:512])
                nc.vector.tensor_mul(ot_sb[64:128, m, qs],
                                     ot_ps[64:128, 512:1024],
                                     rec[0:64, 512:1024])
                if m == M2 - 1:
                    idx2 = 0
                    for st in range(q * 4, q * 4 + 4):
                        for n in range(2):
                            add(i_emit + 2 + 2 * idx2, "y", st, n)
                            idx2 += 1

        emit_proj([("qk", (wk_sb, bkt_sb, kt_sb, 0, 0)),   # KT m0 chunk 0
                   ("qk", (wq_sb, bqt_sb, qt_sb, 0, 0))])  # QT m0 q0
        stp_cur = st_mms(*seq[0])
        emit_proj([("v", (0,))])
        emit_proj([("v", (1,))])
        LAG = 2
        pts = []
        for i, (q, m, kt) in enumerate(seq):
            pt = pt_pool.tile([P, 1024], DT, name="pt")
            nc.scalar.activation(
                pt[:], stp_cur[:],
                mybir.ActivationFunctionType.Exp,
                bias=mb_sb[:, kt:kt + 1],
                scale=SCALE,
            )
            pts.append(pt)
            if i + 1 < len(seq):
                stp_next = st_mms(*seq[i + 1])
            if i >= LAG:
                emit_pv(seq[i - LAG], i, pts[i - LAG])
                pts[i - LAG] = None
            if i == len(seq) - 1:
                # collapse the lag at the end: the PE is about to idle, so
                # the write->read turnaround no longer costs anything
                for j in range(LAG - 1, -1, -1):
                    emit_pv(seq[i - j], i + 1, pts[i - j])
            drain(i)
            stp_cur = stp_next
        drain(10 ** 9)

    nc.compile()
    return nc


def _get_built(dt_name="bfloat16"):
    if dt_name not in _BUILT:
        _BUILT[dt_name] = _build(dt_name)
    return _BUILT[dt_name]


def _prep_core_inputs(c, hidden_states, attention_mask, Wq, bq, Wk, bk, Wv, bv, Wo, bo,
                      np_dt):
    b, g = c // 4, c % 4
    hs = slice(g * HDC, (g + 1) * HDC)
    xtT = hidden_states[b].T.astype(np_dt)          # [H, S]
    # xt[c, p, ko, s'] = X^T[ko*128+p, c*512+s']
    xt = np.ascontiguousarray(
        xtT.reshape(KO, P, NQ, 512).transpose(2, 1, 0, 3))

    def wqkv(W):  # [H, HDC] -> [P, KO, HDC]
        return np.ascontiguousarray(
            W[:, hs].astype(np_dt).reshape(KO, P, HDC).transpose(1, 0, 2))

    mb = np.where(attention_mask[b] == 0, np.float32(-30000.0), np.float32(0.0))
    small = np.concatenate([
        np.ascontiguousarray(bq[hs].reshape(M2, P).T).astype(np.float32),
        np.ascontiguousarray(bk[hs].reshape(M2, P).T).astype(np.float32),
        np.tile(bv[hs].astype(np.float32), (P, 1)),
        np.ascontiguousarray(mb.astype(np.float32).reshape(ST_TILES, P).T),
    ], axis=1)
    return {
        "xt": xt,
        "wq": wqkv(Wq),
        "wk": wqkv(Wk),
        "wv": wqkv(Wv),
        "wo": np.ascontiguousarray(
            Wo[hs, :].astype(np_dt).reshape(M2, P, H).transpose(1, 0, 2)),
        "small": np.ascontiguousarray(small),
    }


def kernel(hidden_states, attention_mask, Wq, bq, Wk, bk, Wv, bv, Wo, bo,
           _trace=False, _trace_kwargs=None):
    from concourse.bass_utils import run_bass_kernel_spmd

    hidden_states = np.asarray(hidden_states, np.float32)
    attention_mask = np.asarray(attention_mask)
    Wq, bq = np.asarray(Wq, np.float32), np.asarray(bq, np.float32)
    Wk, bk = np.asarray(Wk, np.float32), np.asarray(bk, np.float32)
    Wv, bv = np.asarray(Wv, np.float32), np.asarray(bv, np.float32)
    Wo, bo = np.asarray(Wo, np.float32), np.asarray(bo, np.float32)

    nc = _get_built()
    np_dt = ml_dtypes.bfloat16
    in_maps = [
        _prep_core_inputs(c, hidden_states, attention_mask,
                          Wq, bq, Wk, bk, Wv, bv, Wo, bo, np_dt)
        for c in range(8)
    ]
    kwargs = {}
    if _trace:
        kwargs["trace"] = True
        if _trace_kwargs:
            kwargs.update(_trace_kwargs)
    res = run_bass_kernel_spmd(nc, in_maps, core_ids=list(range(8)), **kwargs)
    out = np.empty((B, S, H), np.float32)
    for b in range(B):
        acc = res.results[4 * b]["y"].astype(np.float32).copy()
        for c in range(4 * b + 1, 4 * b + 4):
            acc += res.results[c]["y"]
        out[b] = acc + bo[None, :]
    if _trace:
        return out, res
    return out



# revision 18
# speedup vs baseline: 1.0218x; 1.0218x over previous
"""Multi-head attention (B=2, S=2048, H=1024, 16 heads x 64) on 8 trn2 cores.

Sharding: core c handles batch b=c//4 and the 4 heads [4*(c%4) .. 4*(c%4)+3]
(tensor-parallel over the hd=256 column slice of Wq/Wk/Wv and the matching
row slice of Wo).  Each core computes a rank-256 partial of the output
projection for its batch; the host sums the 4 partials per batch and adds bo.

Device kernel (per core, bf16 matmuls with fp32 PSUM accumulate):
  QT[hd,s] = Wq_c^T X_b^T   (lhsT=Wq nat. layout, rhs=X^T prepped on host)
  KT[hd,s] similarly, stored zero-padded to K=128 per head slot (K=64
  matmuls stream ~1.5x slower per column on the PE), V[s,hd] as [ones|V_h].
  Per head pair (2 heads packed in the 128-partition dim):
    ST[k,q]  = [KT_h;0]^T QT_pair    (full K=128, zero rows null the other
                                      head's rows of the shared qt rhs)
    PT       = exp(SCALE*ST + maskbias[k])   (ScalarE, mask folded into bias)
    [d;OT]   = [ones64|V_h]^T PT     (M=128: rows 0:64 d replicated, 64:128 OT
                                      -> no separate denominator matmul)
    OT_norm  = OT * (1/d)            (reciprocal_approx_fast at base
                                      partition 0 + cross-partition DVE mults)
  Y_partial[s,H] = OT_norm^T Wo_c   (fp16 output tiles, host sums partials)

Scheduling: one flat software pipeline; ScalarE's exp paces the attention
stream while the PE runs PV two iterations behind it (avoids the psum
handoff + fresh-SBUF-read latency on pt).  Projection groups (QT/KT/V) and
output tiles drain from a deadline/credit-paced backlog between iterations
so the PE queue stays fed for the whole run.  PSUM pool creation order is
load-bearing (bank-group conflicts between the exp read stream and
concurrent matmul writes; see the pool comments).
"""
import sys

sys.path.insert(0, "/opt/trn_rl_repo")

import numpy as np
import ml_dtypes
from contextlib import ExitStack

B, S, H = 2, 2048, 1024
NH, HD = 16, 64
SCALE = 1.0 / float(np.sqrt(HD))
HPC = 4          # heads per core
HDC = HPC * HD   # 256 per-core head-dim slice
P = 128
KO = H // P      # 8 contraction tiles for the projections
ST_TILES = S // P    # 16
NQ = S // 512        # 4 q-chunks of 512
M2 = HDC // P        # 2 partition-tiles of the per-core head dim

_BUILT = {}


def _build(dt_name="bfloat16"):
    import concourse.bacc as bacc
    import concourse.mybir as mybir
    import concourse.tile as tile

    DT = getattr(mybir.dt, dt_name)
    F32 = mybir.dt.float32

    nc = bacc.Bacc("TRN2", target_bir_lowering=False, debug=False)

    # all inputs pre-rearranged on host so DMAs are per-partition contiguous
    xt_d = nc.dram_tensor("xt", [NQ, P, KO, 512], DT, kind="ExternalInput").ap()
    wq_d = nc.dram_tensor("wq", [P, KO, HDC], DT, kind="ExternalInput").ap()
    wk_d = nc.dram_tensor("wk", [P, KO, HDC], DT, kind="ExternalInput").ap()
    wv_d = nc.dram_tensor("wv", [P, KO, HDC], DT, kind="ExternalInput").ap()
    wo_d = nc.dram_tensor("wo", [P, M2, H], DT, kind="ExternalInput").ap()
    small_d = nc.dram_tensor("small", [P, M2 + M2 + HDC + ST_TILES], F32,
                             kind="ExternalInput").ap()
    F16 = mybir.dt.float16
    y_d = nc.dram_tensor("y", [S, H], F16, kind="ExternalOutput").ap()

    with tile.TileContext(nc) as tc, ExitStack() as ctx:
        consts = ctx.enter_context(tc.tile_pool(name="consts", bufs=1))
        y_pool = ctx.enter_context(tc.tile_pool(name="ysb", bufs=4))
        qkv = ctx.enter_context(tc.tile_pool(name="qkv", bufs=1))
        pt_pool = ctx.enter_context(tc.tile_pool(name="pt", bufs=5))
        sm_pool = ctx.enter_context(tc.tile_pool(name="sm", bufs=4))
        # PSUM bank-group placement matters: this order puts proj in {0,1},
        # the two ST buffers in {2,3}/{4,5} (so the exp read alternates
        # 4-bank groups) and the PV pair in {6,7} -- the measured conflict
        # tax is ~16us; other orders measured up to +40us worse.
        ps_proj = ctx.enter_context(tc.tile_pool(name="ps_proj", bufs=2, space="PSUM"))
        ps_st = ctx.enter_context(tc.tile_pool(name="ps_st", bufs=2, space="PSUM"))
        ps_ot = ctx.enter_context(tc.tile_pool(name="ps_ot", bufs=1, space="PSUM"))

        # ---- engine warmup during the input-DMA window ----
        # ~8 dummy matmuls trip the PE HAM clock-gate (3.4us busy window ->
        # 2.4GHz) and a dummy exp preloads the ScalarE Exp table, both before
        # any real work depends on them.
        ones_sb = consts.tile([P, 64], DT)
        nc.vector.memset(ones_sb[:], 1.0)
        warm_sb = consts.tile([P, 512], DT)
        nc.vector.memset(warm_sb[:], 1.0)
        warm_out = consts.tile([P, 64], DT)
        warm_ps = ps_proj.tile([P, 512], F32, tag="ps", name="warm_ps")
        for _ in range(14):
            nc.tensor.matmul(warm_ps[:], lhsT=warm_sb[:, 0:128], rhs=warm_sb[:],
                             start=True, stop=True)
        nc.scalar.activation(warm_out[:], ones_sb[:],
                             mybir.ActivationFunctionType.Exp,
                             bias=0.0, scale=1.0)

        # ---- input DMAs, strictly criticality-ordered ----
        # The first exp is gated on wk + xt chunk0 -> kt0/qt0 groups -> st0.
        # Those two transfers get the two fastest slots with nothing else
        # competing for HBM: scalar queue runs wk,wq,small,wo; sync queue
        # runs chunk0,wv,chunk1..3.  gpsimd stays empty so it never steals
        # bandwidth from the critical pair (measured: chunk0 landing at
        # ~21us with 3-way competition vs ~13us without).
        wk_sb = consts.tile([P, KO, HDC], DT)
        nc.scalar.dma_start(wk_sb[:], wk_d)
        xt_sb = consts.tile([P, KO, S], DT)
        nc.sync.dma_start(xt_sb[:, :, 0:512], xt_d[0])
        wq_sb = consts.tile([P, KO, HDC], DT)
        nc.scalar.dma_start(wq_sb[:], wq_d)
        wv_sb = consts.tile([P, KO, HDC], DT)
        nc.sync.dma_start(wv_sb[:], wv_d)

        # small per-partition constants ride one packed DMA (dma_start issue
        # cost on the sequencers is ~0.6us each)
        small_sb = consts.tile([P, M2 + M2 + HDC + ST_TILES], F32)
        nc.scalar.dma_start(small_sb[:], small_d)
        bqt_sb = small_sb[:, 0:M2]
        bkt_sb = small_sb[:, M2:2 * M2]
        bvr_sb = small_sb[:, 2 * M2:2 * M2 + HDC]
        mb_sb = small_sb[:, 2 * M2 + HDC:]

        nc.sync.dma_start(xt_sb[:, :, 512:1024], xt_d[1])
        nc.sync.dma_start(xt_sb[:, :, 1024:1536], xt_d[2])
        nc.sync.dma_start(xt_sb[:, :, 1536:2048], xt_d[3])
        wo_sb = consts.tile([P, M2, H], DT)
        nc.scalar.dma_start(wo_sb[:], wo_d)

        qt_sb = qkv.tile([P, M2, S], DT)
        # KT natural layout [hd-pair 128, m, S]: rows 0:64 head 2m, rows
        # 64:128 head 2m+1.  The ST matmuls run as two CONCURRENT K=64
        # row-tiles (tile_position (0,0)/(64,0) auto-derived from the
        # base partitions) -- both heads' score tiles stream in ~512
        # cycles instead of 1024 with the old zero-padded K=128 form.
        kt_sb = qkv.tile([P, M2, S], DT)
        # per head h: cols 0:64 = ones, cols 64:128 = V_h (so the PV lhsT
        # [ones|V_h] computes the softmax denominator replicated across rows
        # 0:64 -- base partition 0, where reciprocal_approx_fast works --
        # and OT in rows 64:128, in the same matmul)
        v_sb = qkv.tile([P, ST_TILES, HPC, P], DT)
        nc.vector.memset(v_sb[:, :, :, 0:HD], 1.0)
        ot_sb = qkv.tile([P, M2, S], DT)

        # ---- projection group emitters ----
        # A spec describes one 8-matmul accumulation group; emitting two specs
        # interleaved lets each group's LDWEIGHTS prefetch under the other
        # group's matmul streaming.
        def _qk_ops(args):
            w_sb, b_sb, out_sb, m, q = args
            qs = slice(q * 512, (q + 1) * 512)
            ps = ps_proj.tile([P, 512], F32, tag="ps", name="ps_qk")

            def mm(ko, start, stop):
                nc.tensor.matmul(
                    ps[:],
                    lhsT=w_sb[:, ko, m * P:(m + 1) * P],
                    rhs=xt_sb[:, ko, qs],
                    start=start, stop=stop,
                )

            def finish():
                nc.vector.tensor_add(
                    out_sb[:, m, qs], ps[:],
                    b_sb[:, m:m + 1].to_broadcast((P, 512)),
                )

            return mm, finish

        def _v_ops(args):
            (st,) = args
            ps_full = ps_proj.tile([P, 512], F32, tag="ps", name="ps_v")
            ps = ps_full[:, :HDC]

            def mm(ko, start, stop):
                nc.tensor.matmul(
                    ps[:],
                    lhsT=xt_sb[:, ko, st * P:(st + 1) * P],
                    rhs=wv_sb[:, ko, :],
                    start=start, stop=stop,
                )

            def finish():
                for h in range(HPC):
                    hs = slice(h * HD, (h + 1) * HD)
                    nc.vector.tensor_add(v_sb[:, st, h, HD:P], ps[:, hs],
                                         bvr_sb[:, hs])

            return mm, finish

        def emit_group(kind, args):
            mm, finish = (_qk_ops if kind == "qk" else _v_ops)(args)
            for ko in range(KO):
                mm(ko, ko == 0, ko == KO - 1)
            finish()

        def emit_proj(specs):
            for kind, args in specs:
                emit_group(kind, args)

        # qk groups drained mid-stream run as two 4-matmul halves in
        # CONSECUTIVE drain pops (the half-group is ~1.1us of PE, which fits
        # the per-iteration slack beside the exp-paced ST/PV stream; a full
        # 2.2us group stalls the next ST and opens an exp gap).  halves of
        # one group are adjacent in the backlog (same deadline, consecutive
        # seq) so no other ps_proj user can slip between them.
        def emit_qk_half(args, half):
            w_sb, b_sb, out_sb, m, q, holder = args
            if half == 0:
                holder[0] = _qk_ops((w_sb, b_sb, out_sb, m, q))
                mm, _ = holder[0]
                for ko in range(4):
                    mm(ko, ko == 0, False)
            else:
                mm, finish = holder[0]
                for ko in range(4, KO):
                    mm(ko, False, ko == KO - 1)
                finish()
                holder[0] = None

        def emit_y_tile(st, n):
            ss = slice(st * P, (st + 1) * P)
            ns = slice(n * 512, (n + 1) * 512)
            yp = ps_proj.tile([P, 512], F32, tag="ps", name="yp")
            for m in range(M2):
                nc.tensor.matmul(
                    yp[:],
                    lhsT=ot_sb[:, m, ss], rhs=wo_sb[:, m, ns],
                    start=(m == 0), stop=(m == M2 - 1),
                )
            y_sb = y_pool.tile([P, 512], F16, name="y_sb")
            # psum->sbuf copies ride the DVE during the exp stream (a ScalarE
            # copy steals ~0.7us from the exp pacer); only the final q-group
            # -- after the last exp -- alternates onto the idle ScalarE so
            # the tail copy chain runs on two engines.
            if st >= 12 and (st * 2 + n) % 2 == 1:
                nc.scalar.activation(y_sb[:], yp[:],
                                     mybir.ActivationFunctionType.Copy)
            else:
                nc.vector.tensor_copy(y_sb[:], yp[:])
            if (st * 2 + n) % 2 == 0:
                nc.sync.dma_start(y_d[ss, ns], y_sb[:])
            else:
                nc.gpsimd.dma_start(y_d[ss, ns], y_sb[:])

        # ---- backlog of work drained through the pipeline ----
        # items: (deadline_iter, seq, kind, args, pe_ns, release); kept
        # sorted by (deadline, seq).  release gates credit-pops: an item
        # never runs before iteration `release` (used to park PE filler for
        # the post-stream tail so HAM stays warm through the final
        # normalize).
        backlog = []
        _seq_no = [0]
        _COST = {"qkA": 1120, "qkB": 1120, "v": 920, "y": 490}

        def add(deadline, kind, *args, release=0):
            backlog.append((deadline, _seq_no[0], kind, args, _COST[kind],
                            release))
            _seq_no[0] += 1

        def add_qk(deadline, w_sb, b_sb, out_sb, m, q):
            holder = [None]
            add(deadline, "qkA", w_sb, b_sb, out_sb, m, q, holder)
            add(deadline + 1, "qkB", w_sb, b_sb, out_sb, m, q, holder)

        def run_item(kind, args):
            if kind == "y":
                emit_y_tile(*args)
            elif kind == "qkA":
                emit_qk_half(args, 0)
            elif kind == "qkB":
                emit_qk_half(args, 1)
            else:
                emit_group(kind, args)

        # discretionary drain pacing: PE has ~500ns/iter spare beside the
        # exp-paced ST/PV stream, but total backlog exceeds total spare, so
        # feed steadily to keep the PE queue non-empty for the whole
        # attention phase instead of exhausting the backlog early and
        # starving the PE late.
        _credit = [0.0]

        def drain(i):
            backlog.sort(key=lambda t: (t[0], t[1]))
            _credit[0] += 500.0
            while backlog:
                dl, _, kind, args, cost, rel = backlog[0]
                if rel > i:
                    break
                if dl <= i + 2 or _credit[0] >= cost:
                    backlog.pop(0)
                    run_item(kind, args)
                    _credit[0] = max(_credit[0] - cost, -2240.0)
                else:
                    break

        # attention group order: q-outer spreads Y work across the pipeline
        seq = [(q, m, kt) for q in range(NQ) for m in range(M2)
               for kt in range(ST_TILES)]
        giter = {}  # (q, m) -> start iter
        for i, (q, m, kt) in enumerate(seq):
            if kt == 0:
                giter[(q, m)] = i

        # prefix: just enough for the pipeline to start (v0/v1 are emitted
        # right after the first ST matmuls below, so the first exp isn't
        # queued behind them)

        # backlog deadlines
        for m in range(M2):
            for j in range(NQ):
                if (m, j) != (0, 0):
                    # KT chunk j needed by kt=4j of every group of this m
                    add_qk(giter[(0, m)] + 4 * j - 1, wk_sb, bkt_sb, kt_sb,
                           m, j)
                if (m, j) != (0, 0):
                    add_qk(giter[(j, m)] - 1, wq_sb, bqt_sb, qt_sb, m, j)
        for st in range(2, ST_TILES):
            add(st, "v", st)

        # ---- flat attention pipeline ----
        def st_mms(q, m, kt):
            ks = slice(kt * P, (kt + 1) * P)
            qs = slice(q * 512, (q + 1) * 512)
            stp = ps_st.tile([P, 1024], F32, name="stp", tag="stp")
            # two concurrent K=64 row-tiles (auto tile_position (0,0) and
            # (64,0)): head A scores to cols 0:512, head B to 512:1024
            nc.tensor.matmul(
                stp[:, 0:512],
                lhsT=kt_sb[0:64, m, ks], rhs=qt_sb[0:64, m, qs],
                start=True, stop=True,
            )
            nc.tensor.matmul(
                stp[:, 512:1024],
                lhsT=kt_sb[64:128, m, ks], rhs=qt_sb[64:128, m, qs],
                start=True, stop=True,
            )
            return stp

        # PV runs two iterations BEHIND the exp pacer: at iter i the PE
        # consumes pt_{i-2}.  With lag 1 the PE (faster per-iter than
        # ScalarE once the backlog drains) catches up and stalls ~160ns per
        # iteration waiting on exp_{i-1} completing plus the fresh SBUF
        # write->read turnaround on pt; lag 2 gives a full exp period of
        # slack.
        _ot_ps = [None]

        def emit_pv(idx, i_emit, pt):
            q, m, kt = idx
            qs = slice(q * 512, (q + 1) * 512)
            hA, hB = 2 * m, 2 * m + 1
            if kt == 0:
                _ot_ps[0] = ps_ot.tile([P, 1024], F32, name="ot_ps")
            ot_ps = _ot_ps[0]
            # [d;OT] accumulation: lhsT [ones|V_h] -> rows 0:64 denominator
            # replicated, rows 64:128 OT_h (head A cols 0:512 of the pair
            # psum, head B cols 512:1024)
            nc.tensor.matmul(
                ot_ps[:, 0:512],
                lhsT=v_sb[:, kt, hA, :], rhs=pt[:, 0:512],
                start=(kt == 0), stop=(kt == ST_TILES - 1),
            )
            nc.tensor.matmul(
                ot_ps[:, 512:1024],
                lhsT=v_sb[:, kt, hB, :], rhs=pt[:, 512:1024],
                start=(kt == 0), stop=(kt == ST_TILES - 1),
            )
            if kt == ST_TILES - 1:
                rec = sm_pool.tile([P, 1024], F32, tag="rec", name="rec")
                nc.vector.reciprocal_approx_fast(rec[0:64, :], ot_ps[0:64, :])
                nc.vector.tensor_mul(ot_sb[0:64, m, qs],
                                     ot_ps[64:128, 0:512],
                                     rec[0:64, 0:512])
                nc.vector.tensor_mul(ot_sb[64:128, m, qs],
                                     ot_ps[64:128, 512:1024],
                                     rec[0:64, 512:1024])
                if m == M2 - 1:
                    idx2 = 0
                    for st in range(q * 4, q * 4 + 4):
                        for n in range(2):
                            if q == NQ - 2 and idx2 >= 4:
                                # park 4 of q2's y tiles as tail filler:
                                # they run right after the last exp, keeping
                                # the PE busy (HAM warm) through the final
                                # group's DVE normalize so q3's y matmuls
                                # run at 2.4GHz instead of re-throttled
                                add(124 + idx2, "y", st, n, release=126)
                            else:
                                add(i_emit + 2 + 2 * idx2, "y", st, n)
                            idx2 += 1

        emit_proj([("qk", (wk_sb, bkt_sb, kt_sb, 0, 0)),   # KT m0 chunk 0
                   ("qk", (wq_sb, bqt_sb, qt_sb, 0, 0))])  # QT m0 q0
        stp_cur = st_mms(*seq[0])
        emit_proj([("v", (0,))])
        emit_proj([("v", (1,))])
        LAG = 2
        pts = []
        for i, (q, m, kt) in enumerate(seq):
            pt = pt_pool.tile([P, 1024], DT, name="pt")
            nc.scalar.activation(
                pt[:], stp_cur[:],
                mybir.ActivationFunctionType.Exp,
                bias=mb_sb[:, kt:kt + 1],
                scale=SCALE,
            )
            pts.append(pt)
            if i + 1 < len(seq):
                stp_next = st_mms(*seq[i + 1])
            if i >= LAG:
                emit_pv(seq[i - LAG], i, pts[i - LAG])
                pts[i - LAG] = None
            if i == len(seq) - 1:
                # collapse the lag at the end: the PE is about to idle, so
                # the write->read turnaround no longer costs anything
                for j in range(LAG - 1, -1, -1):
                    emit_pv(seq[i - j], i + 1, pts[i - j])
            drain(i)
            stp_cur = stp_next
        drain(10 ** 9)

    nc.compile()
    return nc


def _get_built(dt_name="bfloat16"):
    if dt_name not in _BUILT:
        _BUILT[dt_name] = _build(dt_name)
    return _BUILT[dt_name]


def _prep_core_inputs(c, hidden_states, attention_mask, Wq, bq, Wk, bk, Wv, bv, Wo, bo,
                      np_dt):
    b, g = c // 4, c % 4
    hs = slice(g * HDC, (g + 1) * HDC)
    xtT = hidden_states[b].T.astype(np_dt)          # [H, S]
    # xt[c, p, ko, s'] = X^T[ko*128+p, c*512+s']
    xt = np.ascontiguousarray(
        xtT.reshape(KO, P, NQ, 512).transpose(2, 1, 0, 3))

    def wqkv(W):  # [H, HDC] -> [P, KO, HDC]
        return np.ascontiguousarray(
            W[:, hs].astype(np_dt).reshape(KO, P, HDC).transpose(1, 0, 2))

    mb = np.where(attention_mask[b] == 0, np.float32(-30000.0), np.float32(0.0))
    small = np.concatenate([
        np.ascontiguousarray(bq[hs].reshape(M2, P).T).astype(np.float32),
        np.ascontiguousarray(bk[hs].reshape(M2, P).T).astype(np.float32),
        np.tile(bv[hs].astype(np.float32), (P, 1)),
        np.ascontiguousarray(mb.astype(np.float32).reshape(ST_TILES, P).T),
    ], axis=1)
    return {
        "xt": xt,
        "wq": wqkv(Wq),
        "wk": wqkv(Wk),
        "wv": wqkv(Wv),
        "wo": np.ascontiguousarray(
            Wo[hs, :].astype(np_dt).reshape(M2, P, H).transpose(1, 0, 2)),
        "small": np.ascontiguousarray(small),
    }


def kernel(hidden_states, attention_mask, Wq, bq, Wk, bk, Wv, bv, Wo, bo,
           _trace=False, _trace_kwargs=None):
    from concourse.bass_utils import run_bass_kernel_spmd

    hidden_states = np.asarray(hidden_states, np.float32)
    attention_mask = np.asarray(attention_mask)
    Wq, bq = np.asarray(Wq, np.float32), np.asarray(bq, np.float32)
    Wk, bk = np.asarray(Wk, np.float32), np.asarray(bk, np.float32)
    Wv, bv = np.asarray(Wv, np.float32), np.asarray(bv, np.float32)
    Wo, bo = np.asarray(Wo, np.float32), np.asarray(bo, np.float32)

    nc = _get_built()
    np_dt = ml_dtypes.bfloat16
    in_maps = [
        _prep_core_inputs(c, hidden_states, attention_mask,
                          Wq, bq, Wk, bk, Wv, bv, Wo, bo, np_dt)
        for c in range(8)
    ]
    kwargs = {}
    if _trace:
        kwargs["trace"] = True
        if _trace_kwargs:
            kwargs.update(_trace_kwargs)
    res = run_bass_kernel_spmd(nc, in_maps, core_ids=list(range(8)), **kwargs)
    out = np.empty((B, S, H), np.float32)
    for b in range(B):
        acc = res.results[4 * b]["y"].astype(np.float32).copy()
        for c in range(4 * b + 1, 4 * b + 4):
            acc += res.results[c]["y"]
        out[b] = acc + bo[None, :]
    if _trace:
        return out, res
    return out



# revision 21
# speedup vs baseline: 1.0290x; 1.0070x over previous
"""Multi-head attention (B=2, S=2048, H=1024, 16 heads x 64) on 8 trn2 cores.

Sharding: core c handles batch b=c//4 and the 4 heads [4*(c%4) .. 4*(c%4)+3]
(tensor-parallel over the hd=256 column slice of Wq/Wk/Wv and the matching
row slice of Wo).  Each core computes a rank-256 partial of the output
projection for its batch; the host sums the 4 partials per batch and adds bo.

Device kernel (per core, bf16 matmuls with fp32 PSUM accumulate):
  QT[hd,s] = Wq_c^T X_b^T   (lhsT=Wq nat. layout, rhs=X^T prepped on host)
  KT[hd,s] similarly, stored zero-padded to K=128 per head slot (K=64
  matmuls stream ~1.5x slower per column on the PE), V[s,hd] as [ones|V_h].
  Per head pair (2 heads packed in the 128-partition dim):
    ST[k,q]  = [KT_h;0]^T QT_pair    (full K=128, zero rows null the other
                                      head's rows of the shared qt rhs)
    PT       = exp(SCALE*ST + maskbias[k])   (ScalarE, mask folded into bias)
    [d;OT]   = [ones64|V_h]^T PT     (M=128: rows 0:64 d replicated, 64:128 OT
                                      -> no separate denominator matmul)
    OT_norm  = OT * (1/d)            (reciprocal_approx_fast at base
                                      partition 0 + cross-partition DVE mults)
  Y_partial[s,H] = OT_norm^T Wo_c   (fp16 output tiles, host sums partials)

Scheduling: one flat software pipeline; ScalarE's exp paces the attention
stream while the PE runs PV two iterations behind it (avoids the psum
handoff + fresh-SBUF-read latency on pt).  Projection groups (QT/KT/V) and
output tiles drain from a deadline/credit-paced backlog between iterations
so the PE queue stays fed for the whole run.  PSUM pool creation order is
load-bearing (bank-group conflicts between the exp read stream and
concurrent matmul writes; see the pool comments).
"""
import sys

sys.path.insert(0, "/opt/trn_rl_repo")

import numpy as np
import ml_dtypes
from contextlib import ExitStack

B, S, H = 2, 2048, 1024
NH, HD = 16, 64
SCALE = 1.0 / float(np.sqrt(HD))
HPC = 4          # heads per core
HDC = HPC * HD   # 256 per-core head-dim slice
P = 128
KO = H // P      # 8 contraction tiles for the projections
ST_TILES = S // P    # 16
NQ = S // 512        # 4 q-chunks of 512
M2 = HDC // P        # 2 partition-tiles of the per-core head dim

_BUILT = {}


def _build(dt_name="bfloat16"):
    import concourse.bacc as bacc
    import concourse.mybir as mybir
    import concourse.tile as tile

    DT = getattr(mybir.dt, dt_name)
    F32 = mybir.dt.float32

    nc = bacc.Bacc("TRN2", target_bir_lowering=False, debug=False)

    # all inputs pre-rearranged on host so DMAs are per-partition contiguous
    xt_d = nc.dram_tensor("xt", [NQ, P, KO, 512], DT, kind="ExternalInput").ap()
    wq_d = nc.dram_tensor("wq", [P, KO, HDC], DT, kind="ExternalInput").ap()
    wk_d = nc.dram_tensor("wk", [P, KO, HDC], DT, kind="ExternalInput").ap()
    wv_d = nc.dram_tensor("wv", [P, KO, HDC], DT, kind="ExternalInput").ap()
    wo_d = nc.dram_tensor("wo", [P, M2, H], DT, kind="ExternalInput").ap()
    small_d = nc.dram_tensor("small", [P, M2 + M2 + HDC + ST_TILES], F32,
                             kind="ExternalInput").ap()
    F16 = mybir.dt.float16
    y_d = nc.dram_tensor("y", [S, H], F16, kind="ExternalOutput").ap()

    with tile.TileContext(nc) as tc, ExitStack() as ctx:
        consts = ctx.enter_context(tc.tile_pool(name="consts", bufs=1))
        y_pool = ctx.enter_context(tc.tile_pool(name="ysb", bufs=4))
        qkv = ctx.enter_context(tc.tile_pool(name="qkv", bufs=1))
        pt_pool = ctx.enter_context(tc.tile_pool(name="pt", bufs=5))
        sm_pool = ctx.enter_context(tc.tile_pool(name="sm", bufs=4))
        # PSUM bank-group placement matters: this order puts proj in {0,1},
        # the two ST buffers in {2,3}/{4,5} (so the exp read alternates
        # 4-bank groups) and the PV pair in {6,7} -- the measured conflict
        # tax is ~16us; other orders measured up to +40us worse.
        ps_proj = ctx.enter_context(tc.tile_pool(name="ps_proj", bufs=2, space="PSUM"))
        ps_st = ctx.enter_context(tc.tile_pool(name="ps_st", bufs=2, space="PSUM"))
        ps_ot = ctx.enter_context(tc.tile_pool(name="ps_ot", bufs=1, space="PSUM"))

        # ---- engine warmup during the input-DMA window ----
        # ~8 dummy matmuls trip the PE HAM clock-gate (3.4us busy window ->
        # 2.4GHz) and a dummy exp preloads the ScalarE Exp table, both before
        # any real work depends on them.
        ones_sb = consts.tile([P, 64], DT)
        nc.vector.memset(ones_sb[:], 1.0)
        warm_sb = consts.tile([P, 512], DT)
        nc.vector.memset(warm_sb[:], 1.0)
        warm_out = consts.tile([P, 64], DT)
        warm_ps = ps_proj.tile([P, 512], F32, tag="ps", name="warm_ps")
        for _ in range(14):
            nc.tensor.matmul(warm_ps[:], lhsT=warm_sb[:, 0:128], rhs=warm_sb[:],
                             start=True, stop=True)
        nc.scalar.activation(warm_out[:], ones_sb[:],
                             mybir.ActivationFunctionType.Exp,
                             bias=0.0, scale=1.0)

        # ---- input DMAs, strictly criticality-ordered ----
        # The first exp is gated on wk + xt chunk0 -> kt0/qt0 groups -> st0.
        # Those two transfers get the two fastest slots with nothing else
        # competing for HBM: scalar queue runs wk,wq,small,wo; sync queue
        # runs chunk0,wv,chunk1..3.  gpsimd stays empty so it never steals
        # bandwidth from the critical pair (measured: chunk0 landing at
        # ~21us with 3-way competition vs ~13us without).
        # wk and xt chunk0 split in halves: the kt0 group's first 4 matmuls
        # start as soon as the first halves land (~2us earlier than waiting
        # for the full transfers)
        wk_sb = consts.tile([P, KO, HDC], DT)
        nc.scalar.dma_start(wk_sb[:, 0:4], wk_d[:, 0:4])
        xt_sb = consts.tile([P, KO, S], DT)
        nc.sync.dma_start(xt_sb[:, 0:4, 0:512], xt_d[0][:, 0:4])
        nc.scalar.dma_start(wk_sb[:, 4:8], wk_d[:, 4:8])
        nc.sync.dma_start(xt_sb[:, 4:8, 0:512], xt_d[0][:, 4:8])
        wq_sb = consts.tile([P, KO, HDC], DT)
        nc.scalar.dma_start(wq_sb[:], wq_d)
        wv_sb = consts.tile([P, KO, HDC], DT)
        nc.sync.dma_start(wv_sb[:], wv_d)

        # small per-partition constants ride one packed DMA (dma_start issue
        # cost on the sequencers is ~0.6us each)
        small_sb = consts.tile([P, M2 + M2 + HDC + ST_TILES], F32)
        nc.scalar.dma_start(small_sb[:], small_d)
        bqt_sb = small_sb[:, 0:M2]
        bkt_sb = small_sb[:, M2:2 * M2]
        bvr_sb = small_sb[:, 2 * M2:2 * M2 + HDC]
        mb_sb = small_sb[:, 2 * M2 + HDC:]

        nc.sync.dma_start(xt_sb[:, :, 512:1024], xt_d[1])
        nc.sync.dma_start(xt_sb[:, :, 1024:1536], xt_d[2])
        nc.sync.dma_start(xt_sb[:, :, 1536:2048], xt_d[3])
        wo_sb = consts.tile([P, M2, H], DT)
        nc.scalar.dma_start(wo_sb[:], wo_d)

        qt_sb = qkv.tile([P, M2, S], DT)
        # KT natural layout [hd-pair 128, m, S]: rows 0:64 head 2m, rows
        # 64:128 head 2m+1.  The ST matmuls run as two CONCURRENT K=64
        # row-tiles (tile_position (0,0)/(64,0) auto-derived from the
        # base partitions) -- both heads' score tiles stream in ~512
        # cycles instead of 1024 with the old zero-padded K=128 form.
        kt_sb = qkv.tile([P, M2, S], DT)
        # per head h: cols 0:64 = ones, cols 64:128 = V_h (so the PV lhsT
        # [ones|V_h] computes the softmax denominator replicated across rows
        # 0:64 -- base partition 0, where reciprocal_approx_fast works --
        # and OT in rows 64:128, in the same matmul)
        v_sb = qkv.tile([P, ST_TILES, HPC, P], DT)
        nc.vector.memset(v_sb[:, :, :, 0:HD], 1.0)
        ot_sb = qkv.tile([P, M2, S], DT)

        # ---- projection group emitters ----
        # A spec describes one 8-matmul accumulation group; emitting two specs
        # interleaved lets each group's LDWEIGHTS prefetch under the other
        # group's matmul streaming.
        def _qk_ops(args):
            w_sb, b_sb, out_sb, m, q = args
            qs = slice(q * 512, (q + 1) * 512)
            ps = ps_proj.tile([P, 512], F32, tag="ps", name="ps_qk")

            def mm(ko, start, stop):
                nc.tensor.matmul(
                    ps[:],
                    lhsT=w_sb[:, ko, m * P:(m + 1) * P],
                    rhs=xt_sb[:, ko, qs],
                    start=start, stop=stop,
                )

            def finish():
                nc.vector.tensor_add(
                    out_sb[:, m, qs], ps[:],
                    b_sb[:, m:m + 1].to_broadcast((P, 512)),
                )

            return mm, finish

        def _v_ops(args):
            (st,) = args
            ps_full = ps_proj.tile([P, 512], F32, tag="ps", name="ps_v")
            ps = ps_full[:, :HDC]

            def mm(ko, start, stop):
                nc.tensor.matmul(
                    ps[:],
                    lhsT=xt_sb[:, ko, st * P:(st + 1) * P],
                    rhs=wv_sb[:, ko, :],
                    start=start, stop=stop,
                )

            def finish():
                for h in range(HPC):
                    hs = slice(h * HD, (h + 1) * HD)
                    nc.vector.tensor_add(v_sb[:, st, h, HD:P], ps[:, hs],
                                         bvr_sb[:, hs])

            return mm, finish

        def emit_group(kind, args):
            mm, finish = (_qk_ops if kind == "qk" else _v_ops)(args)
            for ko in range(KO):
                mm(ko, ko == 0, ko == KO - 1)
            finish()

        def emit_proj(specs):
            for kind, args in specs:
                emit_group(kind, args)

        # qk groups drained mid-stream run as two 4-matmul halves in
        # CONSECUTIVE drain pops (the half-group is ~1.1us of PE, which fits
        # the per-iteration slack beside the exp-paced ST/PV stream; a full
        # 2.2us group stalls the next ST and opens an exp gap).  halves of
        # one group are adjacent in the backlog (same deadline, consecutive
        # seq) so no other ps_proj user can slip between them.
        def emit_qk_half(args, half):
            w_sb, b_sb, out_sb, m, q, holder = args
            if half == 0:
                holder[0] = _qk_ops((w_sb, b_sb, out_sb, m, q))
                mm, _ = holder[0]
                for ko in range(4):
                    mm(ko, ko == 0, False)
            else:
                mm, finish = holder[0]
                for ko in range(4, KO):
                    mm(ko, False, ko == KO - 1)
                finish()
                holder[0] = None

        def emit_y_tile(st, n):
            ss = slice(st * P, (st + 1) * P)
            ns = slice(n * 512, (n + 1) * 512)
            yp = ps_proj.tile([P, 512], F32, tag="ps", name="yp")
            for m in range(M2):
                nc.tensor.matmul(
                    yp[:],
                    lhsT=ot_sb[:, m, ss], rhs=wo_sb[:, m, ns],
                    start=(m == 0), stop=(m == M2 - 1),
                )
            y_sb = y_pool.tile([P, 512], F16, name="y_sb")
            # psum->sbuf copies ride the DVE during the exp stream (a ScalarE
            # copy steals ~0.7us from the exp pacer); only the final q-group
            # -- after the last exp -- alternates onto the idle ScalarE so
            # the tail copy chain runs on two engines.
            if st >= 12 and (st * 2 + n) % 2 == 1:
                nc.scalar.activation(y_sb[:], yp[:],
                                     mybir.ActivationFunctionType.Copy)
            else:
                nc.vector.tensor_copy(y_sb[:], yp[:])
            if (st * 2 + n) % 2 == 0:
                nc.sync.dma_start(y_d[ss, ns], y_sb[:])
            else:
                nc.gpsimd.dma_start(y_d[ss, ns], y_sb[:])

        # ---- backlog of work drained through the pipeline ----
        # items: (deadline_iter, seq, kind, args, pe_ns, release); kept
        # sorted by (deadline, seq).  release gates credit-pops: an item
        # never runs before iteration `release` (used to park PE filler for
        # the post-stream tail so HAM stays warm through the final
        # normalize).
        backlog = []
        _seq_no = [0]
        _COST = {"qkA": 1120, "qkB": 1120, "v": 920, "y": 490}

        def add(deadline, kind, *args, release=0):
            backlog.append((deadline, _seq_no[0], kind, args, _COST[kind],
                            release))
            _seq_no[0] += 1

        def add_qk(deadline, w_sb, b_sb, out_sb, m, q):
            holder = [None]
            add(deadline, "qkA", w_sb, b_sb, out_sb, m, q, holder)
            add(deadline + 1, "qkB", w_sb, b_sb, out_sb, m, q, holder)

        def run_item(kind, args):
            if kind == "y":
                emit_y_tile(*args)
            elif kind == "qkA":
                emit_qk_half(args, 0)
            elif kind == "qkB":
                emit_qk_half(args, 1)
            else:
                emit_group(kind, args)

        # discretionary drain pacing: PE has ~500ns/iter spare beside the
        # exp-paced ST/PV stream, but total backlog exceeds total spare, so
        # feed steadily to keep the PE queue non-empty for the whole
        # attention phase instead of exhausting the backlog early and
        # starving the PE late.
        _credit = [0.0]

        def drain(i):
            backlog.sort(key=lambda t: (t[0], t[1]))
            _credit[0] += 500.0
            while backlog:
                dl, _, kind, args, cost, rel = backlog[0]
                if rel > i:
                    break
                if dl <= i + 2 or _credit[0] >= cost:
                    backlog.pop(0)
                    run_item(kind, args)
                    _credit[0] = max(_credit[0] - cost, -2240.0)
                else:
                    break

        # attention group order: q-outer spreads Y work across the pipeline
        seq = [(q, m, kt) for q in range(NQ) for m in range(M2)
               for kt in range(ST_TILES)]
        giter = {}  # (q, m) -> start iter
        for i, (q, m, kt) in enumerate(seq):
            if kt == 0:
                giter[(q, m)] = i

        # prefix: just enough for the pipeline to start (v0/v1 are emitted
        # right after the first ST matmuls below, so the first exp isn't
        # queued behind them)

        # backlog deadlines
        for m in range(M2):
            for j in range(NQ):
                if (m, j) != (0, 0):
                    # KT chunk j needed by kt=4j of every group of this m
                    add_qk(giter[(0, m)] + 4 * j - 1, wk_sb, bkt_sb, kt_sb,
                           m, j)
                if (m, j) != (0, 0):
                    # -3: the finishing DVE bias-add must land a couple of
                    # iterations before the group boundary's first ST reads
                    # qt, else the in-order PE queue bubbles ~0.6us
                    add_qk(giter[(j, m)] - 3, wq_sb, bqt_sb, qt_sb, m, j)
        for st in range(2, ST_TILES):
            add(st, "v", st)

        # ---- flat attention pipeline ----
        def st_mms(q, m, kt):
            ks = slice(kt * P, (kt + 1) * P)
            qs = slice(q * 512, (q + 1) * 512)
            stp = ps_st.tile([P, 1024], F32, name="stp", tag="stp")
            # two concurrent K=64 row-tiles (auto tile_position (0,0) and
            # (64,0)): head A scores to cols 0:512, head B to 512:1024
            nc.tensor.matmul(
                stp[:, 0:512],
                lhsT=kt_sb[0:64, m, ks], rhs=qt_sb[0:64, m, qs],
                start=True, stop=True,
            )
            nc.tensor.matmul(
                stp[:, 512:1024],
                lhsT=kt_sb[64:128, m, ks], rhs=qt_sb[64:128, m, qs],
                start=True, stop=True,
            )
            return stp

        # PV runs two iterations BEHIND the exp pacer: at iter i the PE
        # consumes pt_{i-2}.  With lag 1 the PE (faster per-iter than
        # ScalarE once the backlog drains) catches up and stalls ~160ns per
        # iteration waiting on exp_{i-1} completing plus the fresh SBUF
        # write->read turnaround on pt; lag 2 gives a full exp period of
        # slack.
        _ot_ps = [None]

        def emit_pv(idx, i_emit, pt):
            q, m, kt = idx
            qs = slice(q * 512, (q + 1) * 512)
            hA, hB = 2 * m, 2 * m + 1
            if kt == 0:
                _ot_ps[0] = ps_ot.tile([P, 1024], F32, name="ot_ps")
            ot_ps = _ot_ps[0]
            # [d;OT] accumulation: lhsT [ones|V_h] -> rows 0:64 denominator
            # replicated, rows 64:128 OT_h (head A cols 0:512 of the pair
            # psum, head B cols 512:1024)
            nc.tensor.matmul(
                ot_ps[:, 0:512],
                lhsT=v_sb[:, kt, hA, :], rhs=pt[:, 0:512],
                start=(kt == 0), stop=(kt == ST_TILES - 1),
            )
            nc.tensor.matmul(
                ot_ps[:, 512:1024],
                lhsT=v_sb[:, kt, hB, :], rhs=pt[:, 512:1024],
                start=(kt == 0), stop=(kt == ST_TILES - 1),
            )
            if kt == ST_TILES - 1:
                rec = sm_pool.tile([P, 1024], F32, tag="rec", name="rec")
                nc.vector.reciprocal_approx_fast(rec[0:64, :], ot_ps[0:64, :])
                nc.vector.tensor_mul(ot_sb[0:64, m, qs],
                                     ot_ps[64:128, 0:512],
                                     rec[0:64, 0:512])
                nc.vector.tensor_mul(ot_sb[64:128, m, qs],
                                     ot_ps[64:128, 512:1024],
                                     rec[0:64, 512:1024])
                if m == M2 - 1:
                    idx2 = 0
                    for st in range(q * 4, q * 4 + 4):
                        for n in range(2):
                            if q == NQ - 2 and idx2 >= 4:
                                # park 4 of q2's y tiles as tail filler:
                                # they run right after the last exp, keeping
                                # the PE busy (HAM warm) through the final
                                # group's DVE normalize so q3's y matmuls
                                # run at 2.4GHz instead of re-throttled
                                add(124 + idx2, "y", st, n, release=127)
                            else:
                                add(i_emit + 2 + 2 * idx2, "y", st, n)
                            idx2 += 1

        emit_proj([("qk", (wk_sb, bkt_sb, kt_sb, 0, 0)),   # KT m0 chunk 0
                   ("qk", (wq_sb, bqt_sb, qt_sb, 0, 0))])  # QT m0 q0
        stp_cur = st_mms(*seq[0])
        emit_proj([("v", (0,))])
        emit_proj([("v", (1,))])
        LAG = 2
        pts = []
        for i, (q, m, kt) in enumerate(seq):
            pt = pt_pool.tile([P, 1024], DT, name="pt")
            nc.scalar.activation(
                pt[:], stp_cur[:],
                mybir.ActivationFunctionType.Exp,
                bias=mb_sb[:, kt:kt + 1],
                scale=SCALE,
            )
            pts.append(pt)
            if i + 1 < len(seq):
                stp_next = st_mms(*seq[i + 1])
            if i >= LAG:
                emit_pv(seq[i - LAG], i, pts[i - LAG])
                pts[i - LAG] = None
            if i == len(seq) - 1:
                # collapse the lag at the end: the PE is about to idle, so
                # the write->read turnaround no longer costs anything
                for j in range(LAG - 1, -1, -1):
                    emit_pv(seq[i - j], i + 1, pts[i - j])
            drain(i)
            stp_cur = stp_next
        drain(10 ** 9)

    nc.compile()
    return nc


def _get_built(dt_name="bfloat16"):
    if dt_name not in _BUILT:
        _BUILT[dt_name] = _build(dt_name)
    return _BUILT[dt_name]


def _prep_core_inputs(c, hidden_states, attention_mask, Wq, bq, Wk, bk, Wv, bv, Wo, bo,
                      np_dt):
    b, g = c // 4, c % 4
    hs = slice(g * HDC, (g + 1) * HDC)
    xtT = hidden_states[b].T.astype(np_dt)          # [H, S]
    # xt[c, p, ko, s'] = X^T[ko*128+p, c*512+s']
    xt = np.ascontiguousarray(
        xtT.reshape(KO, P, NQ, 512).transpose(2, 1, 0, 3))

    def wqkv(W):  # [H, HDC] -> [P, KO, HDC]
        return np.ascontiguousarray(
            W[:, hs].astype(np_dt).reshape(KO, P, HDC).transpose(1, 0, 2))

    mb = np.where(attention_mask[b] == 0, np.float32(-30000.0), np.float32(0.0))
    small = np.concatenate([
        np.ascontiguousarray(bq[hs].reshape(M2, P).T).astype(np.float32),
        np.ascontiguousarray(bk[hs].reshape(M2, P).T).astype(np.float32),
        np.tile(bv[hs].astype(np.float32), (P, 1)),
        np.ascontiguousarray(mb.astype(np.float32).reshape(ST_TILES, P).T),
    ], axis=1)
    return {
        "xt": xt,
        "wq": wqkv(Wq),
        "wk": wqkv(Wk),
        "wv": wqkv(Wv),
        "wo": np.ascontiguousarray(
            Wo[hs, :].astype(np_dt).reshape(M2, P, H).transpose(1, 0, 2)),
        "small": np.ascontiguousarray(small),
    }


def kernel(hidden_states, attention_mask, Wq, bq, Wk, bk, Wv, bv, Wo, bo,
           _trace=False, _trace_kwargs=None):
    from concourse.bass_utils import run_bass_kernel_spmd

    hidden_states = np.asarray(hidden_states, np.float32)
    attention_mask = np.asarray(attention_mask)
    Wq, bq = np.asarray(Wq, np.float32), np.asarray(bq, np.float32)
    Wk, bk = np.asarray(Wk, np.float32), np.asarray(bk, np.float32)
    Wv, bv = np.asarray(Wv, np.float32), np.asarray(bv, np.float32)
    Wo, bo = np.asarray(Wo, np.float32), np.asarray(bo, np.float32)

    nc = _get_built()
    np_dt = ml_dtypes.bfloat16
    in_maps = [
        _prep_core_inputs(c, hidden_states, attention_mask,
                          Wq, bq, Wk, bk, Wv, bv, Wo, bo, np_dt)
        for c in range(8)
    ]
    kwargs = {}
    if _trace:
        kwargs["trace"] = True
        if _trace_kwargs:
            kwargs.update(_trace_kwargs)
    res = run_bass_kernel_spmd(nc, in_maps, core_ids=list(range(8)), **kwargs)
    out = np.empty((B, S, H), np.float32)
    for b in range(B):
        acc = res.results[4 * b]["y"].astype(np.float32).copy()
        for c in range(4 * b + 1, 4 * b + 4):
            acc += res.results[c]["y"]
        out[b] = acc + bo[None, :]
    if _trace:
        return out, res
    return out



# revision 22
# speedup vs baseline: 1.0347x; 1.0056x over previous
"""Multi-head attention (B=2, S=2048, H=1024, 16 heads x 64) on 8 trn2 cores.

Sharding: core c handles batch b=c//4 and the 4 heads [4*(c%4) .. 4*(c%4)+3]
(tensor-parallel over the hd=256 column slice of Wq/Wk/Wv and the matching
row slice of Wo).  Each core computes a rank-256 partial of the output
projection for its batch; the host sums the 4 partials per batch and adds bo.

Device kernel (per core, bf16 matmuls with fp32 PSUM accumulate):
  QT[hd,s] = Wq_c^T X_b^T   (lhsT=Wq nat. layout, rhs=X^T prepped on host)
  KT[hd,s] similarly, stored zero-padded to K=128 per head slot (K=64
  matmuls stream ~1.5x slower per column on the PE), V[s,hd] as [ones|V_h].
  Per head pair (2 heads packed in the 128-partition dim):
    ST[k,q]  = [KT_h;0]^T QT_pair    (full K=128, zero rows null the other
                                      head's rows of the shared qt rhs)
    PT       = exp(SCALE*ST + maskbias[k])   (ScalarE, mask folded into bias)
    [d;OT]   = [ones64|V_h]^T PT     (M=128: rows 0:64 d replicated, 64:128 OT
                                      -> no separate denominator matmul)
    OT_norm  = OT * (1/d)            (reciprocal_approx_fast at base
                                      partition 0 + cross-partition DVE mults)
  Y_partial[s,H] = OT_norm^T Wo_c   (fp16 output tiles, host sums partials)

Scheduling: one flat software pipeline; ScalarE's exp paces the attention
stream while the PE runs PV two iterations behind it (avoids the psum
handoff + fresh-SBUF-read latency on pt).  Projection groups (QT/KT/V) and
output tiles drain from a deadline/credit-paced backlog between iterations
so the PE queue stays fed for the whole run.  PSUM pool creation order is
load-bearing (bank-group conflicts between the exp read stream and
concurrent matmul writes; see the pool comments).
"""
import sys

sys.path.insert(0, "/opt/trn_rl_repo")

import numpy as np
import ml_dtypes
from contextlib import ExitStack

B, S, H = 2, 2048, 1024
NH, HD = 16, 64
SCALE = 1.0 / float(np.sqrt(HD))
HPC = 4          # heads per core
HDC = HPC * HD   # 256 per-core head-dim slice
P = 128
KO = H // P      # 8 contraction tiles for the projections
ST_TILES = S // P    # 16
NQ = S // 512        # 4 q-chunks of 512
M2 = HDC // P        # 2 partition-tiles of the per-core head dim

_BUILT = {}


def _build(dt_name="bfloat16"):
    import concourse.bacc as bacc
    import concourse.mybir as mybir
    import concourse.tile as tile

    DT = getattr(mybir.dt, dt_name)
    F32 = mybir.dt.float32

    nc = bacc.Bacc("TRN2", target_bir_lowering=False, debug=False)

    # all inputs pre-rearranged on host so DMAs are per-partition contiguous
    xt_d = nc.dram_tensor("xt", [NQ, P, KO, 512], DT, kind="ExternalInput").ap()
    wq_d = nc.dram_tensor("wq", [P, KO, HDC], DT, kind="ExternalInput").ap()
    wk_d = nc.dram_tensor("wk", [P, KO, HDC], DT, kind="ExternalInput").ap()
    wv_d = nc.dram_tensor("wv", [P, KO, HDC], DT, kind="ExternalInput").ap()
    wo_d = nc.dram_tensor("wo", [P, M2, H], DT, kind="ExternalInput").ap()
    small_d = nc.dram_tensor("small", [P, M2 + M2 + HDC + ST_TILES], F32,
                             kind="ExternalInput").ap()
    F16 = mybir.dt.float16
    y_d = nc.dram_tensor("y", [S, H], F16, kind="ExternalOutput").ap()

    with tile.TileContext(nc) as tc, ExitStack() as ctx:
        consts = ctx.enter_context(tc.tile_pool(name="consts", bufs=1))
        y_pool = ctx.enter_context(tc.tile_pool(name="ysb", bufs=4))
        qkv = ctx.enter_context(tc.tile_pool(name="qkv", bufs=1))
        pt_pool = ctx.enter_context(tc.tile_pool(name="pt", bufs=5))
        sm_pool = ctx.enter_context(tc.tile_pool(name="sm", bufs=4))
        # PSUM bank-group placement matters: this order puts proj in {0,1},
        # the two ST buffers in {2,3}/{4,5} (so the exp read alternates
        # 4-bank groups) and the PV pair in {6,7} -- the measured conflict
        # tax is ~16us; other orders measured up to +40us worse.
        ps_proj = ctx.enter_context(tc.tile_pool(name="ps_proj", bufs=2, space="PSUM"))
        ps_st = ctx.enter_context(tc.tile_pool(name="ps_st", bufs=2, space="PSUM"))
        ps_ot = ctx.enter_context(tc.tile_pool(name="ps_ot", bufs=1, space="PSUM"))

        # ---- engine warmup during the input-DMA window ----
        # ~8 dummy matmuls trip the PE HAM clock-gate (3.4us busy window ->
        # 2.4GHz) and a dummy exp preloads the ScalarE Exp table, both before
        # any real work depends on them.
        ones_sb = consts.tile([P, 64], DT)
        nc.vector.memset(ones_sb[:], 1.0)
        warm_sb = consts.tile([P, 512], DT)
        nc.vector.memset(warm_sb[:], 1.0)
        warm_out = consts.tile([P, 64], DT)
        warm_ps = ps_proj.tile([P, 512], F32, tag="ps", name="warm_ps")
        for _ in range(14):
            nc.tensor.matmul(warm_ps[:], lhsT=warm_sb[:, 0:128], rhs=warm_sb[:],
                             start=True, stop=True)
        nc.scalar.activation(warm_out[:], ones_sb[:],
                             mybir.ActivationFunctionType.Exp,
                             bias=0.0, scale=1.0)

        # ---- input DMAs, strictly criticality-ordered ----
        # The first exp is gated on wk + xt chunk0 -> kt0/qt0 groups -> st0.
        # Those two transfers get the two fastest slots with nothing else
        # competing for HBM: scalar queue runs wk,wq,small,wo; sync queue
        # runs chunk0,wv,chunk1..3.  gpsimd stays empty so it never steals
        # bandwidth from the critical pair (measured: chunk0 landing at
        # ~21us with 3-way competition vs ~13us without).
        # wk and xt chunk0 split in halves: the kt0 group's first 4 matmuls
        # start as soon as the first halves land (~2us earlier than waiting
        # for the full transfers)
        wk_sb = consts.tile([P, KO, HDC], DT)
        nc.scalar.dma_start(wk_sb[:, 0:4], wk_d[:, 0:4])
        xt_sb = consts.tile([P, KO, S], DT)
        nc.sync.dma_start(xt_sb[:, 0:4, 0:512], xt_d[0][:, 0:4])
        nc.scalar.dma_start(wk_sb[:, 4:8], wk_d[:, 4:8])
        nc.sync.dma_start(xt_sb[:, 4:8, 0:512], xt_d[0][:, 4:8])
        wq_sb = consts.tile([P, KO, HDC], DT)
        nc.scalar.dma_start(wq_sb[:], wq_d)
        wv_sb = consts.tile([P, KO, HDC], DT)
        nc.sync.dma_start(wv_sb[:], wv_d)

        # small per-partition constants ride one packed DMA (dma_start issue
        # cost on the sequencers is ~0.6us each)
        small_sb = consts.tile([P, M2 + M2 + HDC + ST_TILES], F32)
        nc.scalar.dma_start(small_sb[:], small_d)
        bqt_sb = small_sb[:, 0:M2]
        bkt_sb = small_sb[:, M2:2 * M2]
        bvr_sb = small_sb[:, 2 * M2:2 * M2 + HDC]
        mb_sb = small_sb[:, 2 * M2 + HDC:]

        nc.sync.dma_start(xt_sb[:, :, 512:1024], xt_d[1])
        nc.sync.dma_start(xt_sb[:, :, 1024:1536], xt_d[2])
        nc.sync.dma_start(xt_sb[:, :, 1536:2048], xt_d[3])
        wo_sb = consts.tile([P, M2, H], DT)
        nc.scalar.dma_start(wo_sb[:], wo_d)

        qt_sb = qkv.tile([P, M2, S], DT)
        # KT natural layout [hd-pair 128, m, S]: rows 0:64 head 2m, rows
        # 64:128 head 2m+1.  The ST matmuls run as two CONCURRENT K=64
        # row-tiles (tile_position (0,0)/(64,0) auto-derived from the
        # base partitions) -- both heads' score tiles stream in ~512
        # cycles instead of 1024 with the old zero-padded K=128 form.
        kt_sb = qkv.tile([P, M2, S], DT)
        # per head h: cols 0:64 = ones, cols 64:128 = V_h (so the PV lhsT
        # [ones|V_h] computes the softmax denominator replicated across rows
        # 0:64 -- base partition 0, where reciprocal_approx_fast works --
        # and OT in rows 64:128, in the same matmul)
        v_sb = qkv.tile([P, ST_TILES, HPC, P], DT)
        nc.vector.memset(v_sb[:, :, :, 0:HD], 1.0)
        ot_sb = qkv.tile([P, M2, S], DT)

        # ---- projection group emitters ----
        # A spec describes one 8-matmul accumulation group; emitting two specs
        # interleaved lets each group's LDWEIGHTS prefetch under the other
        # group's matmul streaming.
        def _qk_ops(args):
            w_sb, b_sb, out_sb, m, q = args
            qs = slice(q * 512, (q + 1) * 512)
            ps = ps_proj.tile([P, 512], F32, tag="ps", name="ps_qk")

            def mm(ko, start, stop):
                nc.tensor.matmul(
                    ps[:],
                    lhsT=w_sb[:, ko, m * P:(m + 1) * P],
                    rhs=xt_sb[:, ko, qs],
                    start=start, stop=stop,
                )

            def finish():
                nc.vector.tensor_add(
                    out_sb[:, m, qs], ps[:],
                    b_sb[:, m:m + 1].to_broadcast((P, 512)),
                )

            return mm, finish

        def _v_ops(args):
            (st,) = args
            ps_full = ps_proj.tile([P, 512], F32, tag="ps", name="ps_v")
            ps = ps_full[:, :HDC]

            def mm(ko, start, stop):
                nc.tensor.matmul(
                    ps[:],
                    lhsT=xt_sb[:, ko, st * P:(st + 1) * P],
                    rhs=wv_sb[:, ko, :],
                    start=start, stop=stop,
                )

            def finish():
                for h in range(HPC):
                    hs = slice(h * HD, (h + 1) * HD)
                    nc.vector.tensor_add(v_sb[:, st, h, HD:P], ps[:, hs],
                                         bvr_sb[:, hs])

            return mm, finish

        def emit_group(kind, args):
            mm, finish = (_qk_ops if kind == "qk" else _v_ops)(args)
            for ko in range(KO):
                mm(ko, ko == 0, ko == KO - 1)
            finish()

        def emit_proj(specs):
            for kind, args in specs:
                emit_group(kind, args)

        # qk groups drained mid-stream run as two 4-matmul halves in
        # CONSECUTIVE drain pops (the half-group is ~1.1us of PE, which fits
        # the per-iteration slack beside the exp-paced ST/PV stream; a full
        # 2.2us group stalls the next ST and opens an exp gap).  halves of
        # one group are adjacent in the backlog (same deadline, consecutive
        # seq) so no other ps_proj user can slip between them.
        def emit_qk_half(args, half):
            w_sb, b_sb, out_sb, m, q, holder = args
            if half == 0:
                holder[0] = _qk_ops((w_sb, b_sb, out_sb, m, q))
                mm, _ = holder[0]
                for ko in range(4):
                    mm(ko, ko == 0, False)
            else:
                mm, finish = holder[0]
                for ko in range(4, KO):
                    mm(ko, False, ko == KO - 1)
                finish()
                holder[0] = None

        def emit_y_tile(st, n):
            ss = slice(st * P, (st + 1) * P)
            ns = slice(n * 512, (n + 1) * 512)
            yp = ps_proj.tile([P, 512], F32, tag="ps", name="yp")
            for m in range(M2):
                nc.tensor.matmul(
                    yp[:],
                    lhsT=ot_sb[:, m, ss], rhs=wo_sb[:, m, ns],
                    start=(m == 0), stop=(m == M2 - 1),
                )
            y_sb = y_pool.tile([P, 512], F16, name="y_sb")
            # psum->sbuf copies ride the DVE during the exp stream (a ScalarE
            # copy steals ~0.7us from the exp pacer); only the final q-group
            # -- after the last exp -- alternates onto the idle ScalarE so
            # the tail copy chain runs on two engines.
            if st >= 12 and (st * 2 + n) % 2 == 1:
                nc.scalar.activation(y_sb[:], yp[:],
                                     mybir.ActivationFunctionType.Copy)
            else:
                nc.vector.tensor_copy(y_sb[:], yp[:])
            if (st * 2 + n) % 2 == 0:
                nc.sync.dma_start(y_d[ss, ns], y_sb[:])
            else:
                nc.gpsimd.dma_start(y_d[ss, ns], y_sb[:])

        # ---- backlog of work drained through the pipeline ----
        # items: (deadline_iter, seq, kind, args, pe_ns, release); kept
        # sorted by (deadline, seq).  release gates credit-pops: an item
        # never runs before iteration `release` (used to park PE filler for
        # the post-stream tail so HAM stays warm through the final
        # normalize).
        backlog = []
        _seq_no = [0]
        _COST = {"qkA": 1120, "qkB": 1120, "v": 920, "y": 490}

        def add(deadline, kind, *args, release=0):
            backlog.append((deadline, _seq_no[0], kind, args, _COST[kind],
                            release))
            _seq_no[0] += 1

        def add_qk(deadline, w_sb, b_sb, out_sb, m, q):
            holder = [None]
            add(deadline, "qkA", w_sb, b_sb, out_sb, m, q, holder)
            add(deadline + 1, "qkB", w_sb, b_sb, out_sb, m, q, holder)

        def run_item(kind, args):
            if kind == "y":
                emit_y_tile(*args)
            elif kind == "qkA":
                emit_qk_half(args, 0)
            elif kind == "qkB":
                emit_qk_half(args, 1)
            else:
                emit_group(kind, args)

        # discretionary drain pacing: PE has ~500ns/iter spare beside the
        # exp-paced ST/PV stream, but total backlog exceeds total spare, so
        # feed steadily to keep the PE queue non-empty for the whole
        # attention phase instead of exhausting the backlog early and
        # starving the PE late.
        _credit = [0.0]

        def drain(i):
            backlog.sort(key=lambda t: (t[0], t[1]))
            _credit[0] += 500.0
            while backlog:
                dl, _, kind, args, cost, rel = backlog[0]
                if rel > i:
                    break
                if dl <= i + 2 or _credit[0] >= cost:
                    backlog.pop(0)
                    run_item(kind, args)
                    _credit[0] = max(_credit[0] - cost, -2240.0)
                else:
                    break

        # attention group order: q-outer spreads Y work across the pipeline
        seq = [(q, m, kt) for q in range(NQ) for m in range(M2)
               for kt in range(ST_TILES)]
        giter = {}  # (q, m) -> start iter
        for i, (q, m, kt) in enumerate(seq):
            if kt == 0:
                giter[(q, m)] = i

        # prefix: just enough for the pipeline to start (v0/v1 are emitted
        # right after the first ST matmuls below, so the first exp isn't
        # queued behind them)

        # backlog deadlines
        for m in range(M2):
            for j in range(NQ):
                if (m, j) != (0, 0):
                    # KT chunk j needed by kt=4j of every group of this m
                    add_qk(giter[(0, m)] + 4 * j - 1, wk_sb, bkt_sb, kt_sb,
                           m, j)
                if (m, j) != (0, 0):
                    # -3: the finishing DVE bias-add must land a couple of
                    # iterations before the group boundary's first ST reads
                    # qt, else the in-order PE queue bubbles ~0.6us
                    add_qk(giter[(j, m)] - 3, wq_sb, bqt_sb, qt_sb, m, j)
        for st in range(2, ST_TILES):
            add(st, "v", st)

        # ---- flat attention pipeline ----
        def st_mms(q, m, kt):
            ks = slice(kt * P, (kt + 1) * P)
            qs = slice(q * 512, (q + 1) * 512)
            stp = ps_st.tile([P, 1024], F32, name="stp", tag="stp")
            # two concurrent K=64 row-tiles (auto tile_position (0,0) and
            # (64,0)): head A scores to cols 0:512, head B to 512:1024
            nc.tensor.matmul(
                stp[:, 0:512],
                lhsT=kt_sb[0:64, m, ks], rhs=qt_sb[0:64, m, qs],
                start=True, stop=True,
            )
            nc.tensor.matmul(
                stp[:, 512:1024],
                lhsT=kt_sb[64:128, m, ks], rhs=qt_sb[64:128, m, qs],
                start=True, stop=True,
            )
            return stp

        # PV runs two iterations BEHIND the exp pacer: at iter i the PE
        # consumes pt_{i-2}.  With lag 1 the PE (faster per-iter than
        # ScalarE once the backlog drains) catches up and stalls ~160ns per
        # iteration waiting on exp_{i-1} completing plus the fresh SBUF
        # write->read turnaround on pt; lag 2 gives a full exp period of
        # slack.
        _ot_ps = [None]

        def emit_pv(idx, i_emit, pt):
            q, m, kt = idx
            qs = slice(q * 512, (q + 1) * 512)
            hA, hB = 2 * m, 2 * m + 1
            if kt == 0:
                _ot_ps[0] = ps_ot.tile([P, 1024], F32, name="ot_ps")
            ot_ps = _ot_ps[0]
            # [d;OT] accumulation: lhsT [ones|V_h] -> rows 0:64 denominator
            # replicated, rows 64:128 OT_h (head A cols 0:512 of the pair
            # psum, head B cols 512:1024)
            nc.tensor.matmul(
                ot_ps[:, 0:512],
                lhsT=v_sb[:, kt, hA, :], rhs=pt[:, 0:512],
                start=(kt == 0), stop=(kt == ST_TILES - 1),
            )
            nc.tensor.matmul(
                ot_ps[:, 512:1024],
                lhsT=v_sb[:, kt, hB, :], rhs=pt[:, 512:1024],
                start=(kt == 0), stop=(kt == ST_TILES - 1),
            )
            if kt == ST_TILES - 1:
                rec = sm_pool.tile([P, 1024], F32, tag="rec", name="rec")
                nc.vector.reciprocal_approx_fast(rec[0:64, :], ot_ps[0:64, :])
                nc.vector.tensor_mul(ot_sb[0:64, m, qs],
                                     ot_ps[64:128, 0:512],
                                     rec[0:64, 0:512])
                nc.vector.tensor_mul(ot_sb[64:128, m, qs],
                                     ot_ps[64:128, 512:1024],
                                     rec[0:64, 512:1024])
                if m == M2 - 1:
                    idx2 = 0
                    for st in range(q * 4, q * 4 + 4):
                        for n in range(2):
                            if q == NQ - 2 and idx2 >= 4:
                                # park 4 of q2's y tiles as tail filler:
                                # they run right after the last exp, keeping
                                # the PE busy (HAM warm) through the final
                                # group's DVE normalize so q3's y matmuls
                                # run at 2.4GHz instead of re-throttled
                                add(124 + idx2, "y", st, n, release=127)
                            else:
                                add(i_emit + 2 + 2 * idx2, "y", st, n)
                            idx2 += 1

        def emit_exp(i, stp):
            kt = seq[i][2]
            pt = pt_pool.tile([P, 1024], DT, name="pt")
            nc.scalar.activation(
                pt[:], stp[:],
                mybir.ActivationFunctionType.Exp,
                bias=mb_sb[:, kt:kt + 1],
                scale=SCALE,
            )
            return pt

        emit_proj([("qk", (wk_sb, bkt_sb, kt_sb, 0, 0)),   # KT m0 chunk 0
                   ("qk", (wq_sb, bqt_sb, qt_sb, 0, 0))])  # QT m0 q0
        stps = {0: st_mms(*seq[0])}
        emit_proj([("v", (0,))])
        stps[1] = st_mms(*seq[1])
        emit_proj([("v", (1,))])

        # PAIR-CYCLED pipeline: cycle j covers iterations (2j, 2j+1).
        # Per-cycle PE order: [pv(2j-2) pv(2j-1) drain] (all 128-contraction
        # mode) then [st(2j+2) st(2j+3)] (one 64-row-mode island) -- one
        # tiling-mode flip per cycle instead of two, and by the time the PE
        # works through the ~1.8us pv+drain block, exp(2j) has finished
        # reading the psum buf st(2j+2) wants to overwrite, so the in-order
        # PE queue doesn't stall on the WAR dependency.  PV keeps a lag of
        # one full cycle (2 iterations) behind the exp pacer.
        N = len(seq)
        pts = [None] * N
        for j in range(N // 2):
            i0, i1 = 2 * j, 2 * j + 1
            pts[i0] = emit_exp(i0, stps.pop(i0))
            pts[i1] = emit_exp(i1, stps.pop(i1))
            if j >= 1:
                emit_pv(seq[i0 - 2], i0, pts[i0 - 2])
                pts[i0 - 2] = None
                emit_pv(seq[i1 - 2], i1, pts[i1 - 2])
                pts[i1 - 2] = None
            drain(i1)
            if i1 + 2 < N:
                stps[i0 + 2] = st_mms(*seq[i0 + 2])
                stps[i1 + 2] = st_mms(*seq[i1 + 2])
        # collapse the lag at the end: the PE is about to idle, so the
        # write->read turnaround no longer costs anything
        emit_pv(seq[N - 2], N, pts[N - 2])
        emit_pv(seq[N - 1], N + 1, pts[N - 1])
        drain(10 ** 9)

    nc.compile()
    return nc


def _get_built(dt_name="bfloat16"):
    if dt_name not in _BUILT:
        _BUILT[dt_name] = _build(dt_name)
    return _BUILT[dt_name]


def _prep_core_inputs(c, hidden_states, attention_mask, Wq, bq, Wk, bk, Wv, bv, Wo, bo,
                      np_dt):
    b, g = c // 4, c % 4
    hs = slice(g * HDC, (g + 1) * HDC)
    xtT = hidden_states[b].T.astype(np_dt)          # [H, S]
    # xt[c, p, ko, s'] = X^T[ko*128+p, c*512+s']
    xt = np.ascontiguousarray(
        xtT.reshape(KO, P, NQ, 512).transpose(2, 1, 0, 3))

    def wqkv(W):  # [H, HDC] -> [P, KO, HDC]
        return np.ascontiguousarray(
            W[:, hs].astype(np_dt).reshape(KO, P, HDC).transpose(1, 0, 2))

    mb = np.where(attention_mask[b] == 0, np.float32(-30000.0), np.float32(0.0))
    small = np.concatenate([
        np.ascontiguousarray(bq[hs].reshape(M2, P).T).astype(np.float32),
        np.ascontiguousarray(bk[hs].reshape(M2, P).T).astype(np.float32),
        np.tile(bv[hs].astype(np.float32), (P, 1)),
        np.ascontiguousarray(mb.astype(np.float32).reshape(ST_TILES, P).T),
    ], axis=1)
    return {
        "xt": xt,
        "wq": wqkv(Wq),
        "wk": wqkv(Wk),
        "wv": wqkv(Wv),
        "wo": np.ascontiguousarray(
            Wo[hs, :].astype(np_dt).reshape(M2, P, H).transpose(1, 0, 2)),
        "small": np.ascontiguousarray(small),
    }


def kernel(hidden_states, attention_mask, Wq, bq, Wk, bk, Wv, bv, Wo, bo,
           _trace=False, _trace_kwargs=None):
    from concourse.bass_utils import run_bass_kernel_spmd

    hidden_states = np.asarray(hidden_states, np.float32)
    attention_mask = np.asarray(attention_mask)
    Wq, bq = np.asarray(Wq, np.float32), np.asarray(bq, np.float32)
    Wk, bk = np.asarray(Wk, np.float32), np.asarray(bk, np.float32)
    Wv, bv = np.asarray(Wv, np.float32), np.asarray(bv, np.float32)
    Wo, bo = np.asarray(Wo, np.float32), np.asarray(bo, np.float32)

    nc = _get_built()
    np_dt = ml_dtypes.bfloat16
    in_maps = [
        _prep_core_inputs(c, hidden_states, attention_mask,
                          Wq, bq, Wk, bk, Wv, bv, Wo, bo, np_dt)
        for c in range(8)
    ]
    kwargs = {}
    if _trace:
        kwargs["trace"] = True
        if _trace_kwargs:
            kwargs.update(_trace_kwargs)
    res = run_bass_kernel_spmd(nc, in_maps, core_ids=list(range(8)), **kwargs)
    out = np.empty((B, S, H), np.float32)
    for b in range(B):
        acc = res.results[4 * b]["y"].astype(np.float32).copy()
        for c in range(4 * b + 1, 4 * b + 4):
            acc += res.results[c]["y"]
        out[b] = acc + bo[None, :]
    if _trace:
        return out, res
    return out



# revision 27
# speedup vs baseline: 1.0620x; 1.0263x over previous
"""Multi-head attention (B=2, S=2048, H=1024, 16 heads x 64) on 8 trn2 cores.

Sharding: core c handles batch b=c//4 and the 4 heads [4*(c%4) .. 4*(c%4)+3]
(tensor-parallel over the hd=256 column slice of Wq/Wk/Wv and the matching
row slice of Wo).  Each core computes a rank-256 partial of the output
projection for its batch; the host sums the 4 partials per batch and adds bo.

Device kernel (per core, bf16 matmuls with fp32 PSUM accumulate):
  QT[hd,s] = Wq_c^T X_b^T   (lhsT=Wq nat. layout, rhs=X^T prepped on host)
  KT[hd,s] similarly, stored zero-padded to K=128 per head slot (K=64
  matmuls stream ~1.5x slower per column on the PE), V[s,hd] as [ones|V_h].
  Per head pair (2 heads packed in the 128-partition dim):
    ST[k,q]  = [KT_h;0]^T QT_pair    (full K=128, zero rows null the other
                                      head's rows of the shared qt rhs)
    PT       = exp(SCALE*ST + maskbias[k])   (ScalarE, mask folded into bias)
    [d;OT]   = [ones64|V_h]^T PT     (M=128: rows 0:64 d replicated, 64:128 OT
                                      -> no separate denominator matmul)
    OT_norm  = OT * (1/d)            (reciprocal_approx_fast at base
                                      partition 0 + cross-partition DVE mults)
  Y_partial[s,H] = OT_norm^T Wo_c   (fp16 output tiles, host sums partials)

Scheduling: one flat software pipeline; ScalarE's exp paces the attention
stream while the PE runs PV two iterations behind it (avoids the psum
handoff + fresh-SBUF-read latency on pt).  Projection groups (QT/KT/V) and
output tiles drain from a deadline/credit-paced backlog between iterations
so the PE queue stays fed for the whole run.  PSUM pool creation order is
load-bearing (bank-group conflicts between the exp read stream and
concurrent matmul writes; see the pool comments).
"""
import sys

sys.path.insert(0, "/opt/trn_rl_repo")

import numpy as np
import ml_dtypes
from contextlib import ExitStack

B, S, H = 2, 2048, 1024
NH, HD = 16, 64
SCALE = 1.0 / float(np.sqrt(HD))
HPC = 4          # heads per core
HDC = HPC * HD   # 256 per-core head-dim slice
P = 128
KO = H // P      # 8 contraction tiles for the projections
ST_TILES = S // P    # 16
NQ = S // 512        # 4 q-chunks of 512
M2 = HDC // P        # 2 partition-tiles of the per-core head dim

_BUILT = {}


def _build(dt_name="bfloat16"):
    import concourse.bacc as bacc
    import concourse.mybir as mybir
    import concourse.tile as tile

    DT = getattr(mybir.dt, dt_name)
    F32 = mybir.dt.float32

    nc = bacc.Bacc("TRN2", target_bir_lowering=False, debug=False)

    # all inputs pre-rearranged on host so DMAs are per-partition contiguous
    xt_d = nc.dram_tensor("xt", [NQ, P, KO, 512], DT, kind="ExternalInput").ap()
    wq_d = nc.dram_tensor("wq", [P, KO, HDC], DT, kind="ExternalInput").ap()
    wk_d = nc.dram_tensor("wk", [P, KO, HDC], DT, kind="ExternalInput").ap()
    wv_d = nc.dram_tensor("wv", [P, KO, HDC], DT, kind="ExternalInput").ap()
    wo_d = nc.dram_tensor("wo", [P, M2, H], DT, kind="ExternalInput").ap()
    small_d = nc.dram_tensor("small", [P, M2 + M2 + HDC + ST_TILES], F32,
                             kind="ExternalInput").ap()
    F16 = mybir.dt.float16
    y_d = nc.dram_tensor("y", [S, H], F16, kind="ExternalOutput").ap()

    with tile.TileContext(nc) as tc, ExitStack() as ctx:
        consts = ctx.enter_context(tc.tile_pool(name="consts", bufs=1))
        y_pool = ctx.enter_context(tc.tile_pool(name="ysb", bufs=4))
        qkv = ctx.enter_context(tc.tile_pool(name="qkv", bufs=1))
        pt_pool = ctx.enter_context(tc.tile_pool(name="pt", bufs=7))
        sm_pool = ctx.enter_context(tc.tile_pool(name="sm", bufs=4))
        # PSUM bank-group placement matters: this order puts proj in {0,1},
        # the two ST buffers in {2,3}/{4,5} (so the exp read alternates
        # 4-bank groups) and the PV pair in {6,7} -- the measured conflict
        # tax is ~16us; other orders measured up to +40us worse.
        ps_proj = ctx.enter_context(tc.tile_pool(name="ps_proj", bufs=2, space="PSUM"))
        ps_st = ctx.enter_context(tc.tile_pool(name="ps_st", bufs=2, space="PSUM"))
        ps_ot = ctx.enter_context(tc.tile_pool(name="ps_ot", bufs=1, space="PSUM"))

        # ---- engine warmup during the input-DMA window ----
        # ~8 dummy matmuls trip the PE HAM clock-gate (3.4us busy window ->
        # 2.4GHz) and a dummy exp preloads the ScalarE Exp table, both before
        # any real work depends on them.
        ones_sb = consts.tile([P, 64], DT)
        nc.vector.memset(ones_sb[:], 1.0)
        warm_sb = consts.tile([P, 512], DT)
        nc.vector.memset(warm_sb[:], 1.0)
        warm_out = consts.tile([P, 64], DT)
        warm_ps = ps_proj.tile([P, 512], F32, tag="ps", name="warm_ps")
        for _ in range(18):
            nc.tensor.matmul(warm_ps[:], lhsT=warm_sb[:, 0:128], rhs=warm_sb[:],
                             start=True, stop=True)
        nc.scalar.activation(warm_out[:], ones_sb[:],
                             mybir.ActivationFunctionType.Exp,
                             bias=0.0, scale=1.0)

        # ---- input DMAs, strictly criticality-ordered ----
        # The first exp is gated on wk + xt chunk0 -> kt0/qt0 groups -> st0.
        # Those two transfers get the two fastest slots with nothing else
        # competing for HBM: scalar queue runs wk,wq,small,wo; sync queue
        # runs chunk0,wv,chunk1..3.  gpsimd stays empty so it never steals
        # bandwidth from the critical pair (measured: chunk0 landing at
        # ~21us with 3-way competition vs ~13us without).
        # wk and xt chunk0 split in halves: the kt0 group's first 4 matmuls
        # start as soon as the first halves land (~2us earlier than waiting
        # for the full transfers)
        wk_sb = consts.tile([P, KO, HDC], DT)
        nc.scalar.dma_start(wk_sb[:, 0:4], wk_d[:, 0:4])
        xt_sb = consts.tile([P, KO, S], DT)
        nc.sync.dma_start(xt_sb[:, 0:4, 0:512], xt_d[0][:, 0:4])
        nc.scalar.dma_start(wk_sb[:, 4:8], wk_d[:, 4:8])
        nc.sync.dma_start(xt_sb[:, 4:8, 0:512], xt_d[0][:, 4:8])
        wq_sb = consts.tile([P, KO, HDC], DT)
        nc.scalar.dma_start(wq_sb[:], wq_d)
        wv_sb = consts.tile([P, KO, HDC], DT)
        nc.sync.dma_start(wv_sb[:], wv_d)

        # small per-partition constants ride one packed DMA (dma_start issue
        # cost on the sequencers is ~0.6us each)
        small_sb = consts.tile([P, M2 + M2 + HDC + ST_TILES], F32)
        nc.scalar.dma_start(small_sb[:], small_d)
        bqt_sb = small_sb[:, 0:M2]
        bkt_sb = small_sb[:, M2:2 * M2]
        bvr_sb = small_sb[:, 2 * M2:2 * M2 + HDC]
        mb_sb = small_sb[:, 2 * M2 + HDC:]

        nc.sync.dma_start(xt_sb[:, :, 512:1024], xt_d[1])
        nc.sync.dma_start(xt_sb[:, :, 1024:1536], xt_d[2])
        nc.sync.dma_start(xt_sb[:, :, 1536:2048], xt_d[3])
        wo_sb = consts.tile([P, M2, H], DT)
        nc.scalar.dma_start(wo_sb[:], wo_d)

        qt_sb = qkv.tile([P, M2, S], DT)
        # KT natural layout [hd-pair 128, m, S]: rows 0:64 head 2m, rows
        # 64:128 head 2m+1.  The ST matmuls run as two CONCURRENT K=64
        # row-tiles (tile_position (0,0)/(64,0) auto-derived from the
        # base partitions) -- both heads' score tiles stream in ~512
        # cycles instead of 1024 with the old zero-padded K=128 form.
        kt_sb = qkv.tile([P, M2, S], DT)
        # per head h: cols 0:64 = ones, cols 64:128 = V_h (so the PV lhsT
        # [ones|V_h] computes the softmax denominator replicated across rows
        # 0:64 -- base partition 0, where reciprocal_approx_fast works --
        # and OT in rows 64:128, in the same matmul)
        v_sb = qkv.tile([P, ST_TILES, HPC, P], DT)
        nc.vector.memset(v_sb[:, :, :, 0:HD], 1.0)
        ot_sb = qkv.tile([P, M2, S], DT)

        # ---- projection group emitters ----
        # A spec describes one 8-matmul accumulation group; emitting two specs
        # interleaved lets each group's LDWEIGHTS prefetch under the other
        # group's matmul streaming.
        def _qk_ops(args):
            w_sb, b_sb, out_sb, m, q = args
            qs = slice(q * 512, (q + 1) * 512)
            ps = ps_proj.tile([P, 512], F32, tag="ps", name="ps_qk")

            def mm(ko, start, stop):
                nc.tensor.matmul(
                    ps[:],
                    lhsT=w_sb[:, ko, m * P:(m + 1) * P],
                    rhs=xt_sb[:, ko, qs],
                    start=start, stop=stop,
                )

            def finish():
                nc.vector.tensor_add(
                    out_sb[:, m, qs], ps[:],
                    b_sb[:, m:m + 1].to_broadcast((P, 512)),
                )

            return mm, finish

        def _v_ops(args):
            (st,) = args
            ps_full = ps_proj.tile([P, 512], F32, tag="ps", name="ps_v")
            ps = ps_full[:, :HDC]

            def mm(ko, start, stop):
                nc.tensor.matmul(
                    ps[:],
                    lhsT=xt_sb[:, ko, st * P:(st + 1) * P],
                    rhs=wv_sb[:, ko, :],
                    start=start, stop=stop,
                )

            def finish():
                for h in range(HPC):
                    hs = slice(h * HD, (h + 1) * HD)
                    nc.vector.tensor_add(v_sb[:, st, h, HD:P], ps[:, hs],
                                         bvr_sb[:, hs])

            return mm, finish

        def emit_group(kind, args):
            mm, finish = (_qk_ops if kind == "qk" else _v_ops)(args)
            for ko in range(KO):
                mm(ko, ko == 0, ko == KO - 1)
            finish()

        def emit_proj(specs):
            for kind, args in specs:
                emit_group(kind, args)

        # qk groups drained mid-stream run as two 4-matmul halves in
        # CONSECUTIVE drain pops (the half-group is ~1.1us of PE, which fits
        # the per-iteration slack beside the exp-paced ST/PV stream; a full
        # 2.2us group stalls the next ST and opens an exp gap).  halves of
        # one group are adjacent in the backlog (same deadline, consecutive
        # seq) so no other ps_proj user can slip between them.
        def emit_qk_half(args, half):
            w_sb, b_sb, out_sb, m, q, holder = args
            if half == 0:
                holder[0] = _qk_ops((w_sb, b_sb, out_sb, m, q))
                mm, _ = holder[0]
                for ko in range(4):
                    mm(ko, ko == 0, False)
            else:
                mm, finish = holder[0]
                for ko in range(4, KO):
                    mm(ko, False, ko == KO - 1)
                finish()
                holder[0] = None

        def emit_y_tile(st, n):
            ss = slice(st * P, (st + 1) * P)
            ns = slice(n * 512, (n + 1) * 512)
            yp = ps_proj.tile([P, 512], F32, tag="ps", name="yp")
            for m in range(M2):
                nc.tensor.matmul(
                    yp[:],
                    lhsT=ot_sb[:, m, ss], rhs=wo_sb[:, m, ns],
                    start=(m == 0), stop=(m == M2 - 1),
                )
            y_sb = y_pool.tile([P, 512], F16, name="y_sb")
            # psum->sbuf copies ride the DVE during the exp stream (a ScalarE
            # copy steals ~0.7us from the exp pacer); only the final q-group
            # -- after the last exp -- alternates onto the idle ScalarE so
            # the tail copy chain runs on two engines.
            if st >= 12 and (st * 2 + n) % 2 == 1:
                nc.scalar.activation(y_sb[:], yp[:],
                                     mybir.ActivationFunctionType.Copy)
            else:
                nc.vector.tensor_copy(y_sb[:], yp[:])
            if (st * 2 + n) % 2 == 0:
                nc.sync.dma_start(y_d[ss, ns], y_sb[:])
            else:
                nc.gpsimd.dma_start(y_d[ss, ns], y_sb[:])

        # ---- backlog of work drained through the pipeline ----
        # items: (deadline_iter, seq, kind, args, pe_ns, release); kept
        # sorted by (deadline, seq).  release gates credit-pops: an item
        # never runs before iteration `release` (used to park PE filler for
        # the post-stream tail so HAM stays warm through the final
        # normalize).
        backlog = []
        _seq_no = [0]
        _COST = {"qkA": 1120, "qkB": 1120, "v": 920, "y": 490}

        def add(deadline, kind, *args, release=0):
            backlog.append((deadline, _seq_no[0], kind, args, _COST[kind],
                            release))
            _seq_no[0] += 1

        def add_qk(deadline, w_sb, b_sb, out_sb, m, q):
            holder = [None]
            add(deadline, "qkA", w_sb, b_sb, out_sb, m, q, holder)
            add(deadline + 1, "qkB", w_sb, b_sb, out_sb, m, q, holder)

        def run_item(kind, args):
            if kind == "y":
                emit_y_tile(*args)
            elif kind == "qkA":
                emit_qk_half(args, 0)
            elif kind == "qkB":
                emit_qk_half(args, 1)
            else:
                emit_group(kind, args)

        # discretionary drain pacing: PE has ~500ns/iter spare beside the
        # exp-paced ST/PV stream, but total backlog exceeds total spare, so
        # feed steadily to keep the PE queue non-empty for the whole
        # attention phase instead of exhausting the backlog early and
        # starving the PE late.
        _credit = [0.0]

        def drain(i):
            backlog.sort(key=lambda t: (t[0], t[1]))
            _credit[0] += 500.0
            while backlog:
                dl, _, kind, args, cost, rel = backlog[0]
                if rel > i:
                    break
                if dl <= i + 2 or _credit[0] >= cost:
                    backlog.pop(0)
                    run_item(kind, args)
                    _credit[0] = max(_credit[0] - cost, -2240.0)
                else:
                    break

        # attention group order: q-outer spreads Y work across the pipeline
        seq = [(q, m, kt) for q in range(NQ) for m in range(M2)
               for kt in range(ST_TILES)]
        giter = {}  # (q, m) -> start iter
        for i, (q, m, kt) in enumerate(seq):
            if kt == 0:
                giter[(q, m)] = i

        # prefix: just enough for the pipeline to start (v0/v1 are emitted
        # right after the first ST matmuls below, so the first exp isn't
        # queued behind them)

        # backlog deadlines
        for m in range(M2):
            for j in range(NQ):
                if (m, j) != (0, 0):
                    # KT chunk j needed by kt=4j of every group of this m
                    add_qk(giter[(0, m)] + 4 * j - 1, wk_sb, bkt_sb, kt_sb,
                           m, j)
                if (m, j) != (0, 0):
                    # -3: the finishing DVE bias-add must land a couple of
                    # iterations before the group boundary's first ST reads
                    # qt, else the in-order PE queue bubbles ~0.6us
                    add_qk(giter[(j, m)] - 3, wq_sb, bqt_sb, qt_sb, m, j)
        for st in range(2, ST_TILES):
            add(st, "v", st)

        # ---- flat attention pipeline ----
        def st_mms(q, m, kt):
            ks = slice(kt * P, (kt + 1) * P)
            qs = slice(q * 512, (q + 1) * 512)
            stp = ps_st.tile([P, 1024], F32, name="stp", tag="stp")
            # two concurrent K=64 row-tiles (auto tile_position (0,0) and
            # (64,0)): head A scores to cols 0:512, head B to 512:1024
            nc.tensor.matmul(
                stp[:, 0:512],
                lhsT=kt_sb[0:64, m, ks], rhs=qt_sb[0:64, m, qs],
                start=True, stop=True,
            )
            nc.tensor.matmul(
                stp[:, 512:1024],
                lhsT=kt_sb[64:128, m, ks], rhs=qt_sb[64:128, m, qs],
                start=True, stop=True,
            )
            return stp

        # PV runs two iterations BEHIND the exp pacer: at iter i the PE
        # consumes pt_{i-2}.  With lag 1 the PE (faster per-iter than
        # ScalarE once the backlog drains) catches up and stalls ~160ns per
        # iteration waiting on exp_{i-1} completing plus the fresh SBUF
        # write->read turnaround on pt; lag 2 gives a full exp period of
        # slack.
        _ot_ps = [None]

        def emit_pv(idx, i_emit, pt):
            q, m, kt = idx
            qs = slice(q * 512, (q + 1) * 512)
            hA, hB = 2 * m, 2 * m + 1
            if kt == 0:
                _ot_ps[0] = ps_ot.tile([P, 1024], F32, name="ot_ps")
            ot_ps = _ot_ps[0]
            # [d;OT] accumulation: lhsT [ones|V_h] -> rows 0:64 denominator
            # replicated, rows 64:128 OT_h (head A cols 0:512 of the pair
            # psum, head B cols 512:1024)
            nc.tensor.matmul(
                ot_ps[:, 0:512],
                lhsT=v_sb[:, kt, hA, :], rhs=pt[:, 0:512],
                start=(kt == 0), stop=(kt == ST_TILES - 1),
            )
            nc.tensor.matmul(
                ot_ps[:, 512:1024],
                lhsT=v_sb[:, kt, hB, :], rhs=pt[:, 512:1024],
                start=(kt == 0), stop=(kt == ST_TILES - 1),
            )
            if kt == ST_TILES - 1:
                rec = sm_pool.tile([P, 1024], F32, tag="rec", name="rec")
                nc.vector.reciprocal_approx_fast(rec[0:64, :], ot_ps[0:64, :])
                nc.vector.tensor_mul(ot_sb[0:64, m, qs],
                                     ot_ps[64:128, 0:512],
                                     rec[0:64, 0:512])
                nc.vector.tensor_mul(ot_sb[64:128, m, qs],
                                     ot_ps[64:128, 512:1024],
                                     rec[0:64, 512:1024])
                if m == M2 - 1:
                    idx2 = 0
                    for st in range(q * 4, q * 4 + 4):
                        for n in range(2):
                            if q == NQ - 2 and idx2 >= 4:
                                # park 4 of q2's y tiles as tail filler:
                                # they run right after the last exp, keeping
                                # the PE busy (HAM warm) through the final
                                # group's DVE normalize so q3's y matmuls
                                # run at 2.4GHz instead of re-throttled
                                add(124 + idx2, "y", st, n, release=128)
                            else:
                                add(i_emit + 2 + 2 * idx2, "y", st, n)
                            idx2 += 1

        def emit_exp(i, stp):
            kt = seq[i][2]
            pt = pt_pool.tile([P, 1024], DT, name="pt")
            nc.scalar.activation(
                pt[:], stp[:],
                mybir.ActivationFunctionType.Exp,
                bias=mb_sb[:, kt:kt + 1],
                scale=SCALE,
            )
            return pt

        emit_proj([("qk", (wk_sb, bkt_sb, kt_sb, 0, 0)),   # KT m0 chunk 0
                   ("qk", (wq_sb, bqt_sb, qt_sb, 0, 0))])  # QT m0 q0
        stps = {0: st_mms(*seq[0])}
        emit_proj([("v", (0,))])
        stps[1] = st_mms(*seq[1])
        emit_proj([("v", (1,))])

        # PAIR-CYCLED pipeline: cycle j covers iterations (2j, 2j+1).
        # Per-cycle PE order: [pv(2j-2) pv(2j-1) drain] (all 128-contraction
        # mode) then [st(2j+2) st(2j+3)] (one 64-row-mode island) -- one
        # tiling-mode flip per cycle instead of two, and by the time the PE
        # works through the ~1.8us pv+drain block, exp(2j) has finished
        # reading the psum buf st(2j+2) wants to overwrite, so the in-order
        # PE queue doesn't stall on the WAR dependency.  PV keeps a lag of
        # one full cycle (2 iterations) behind the exp pacer.
        N = len(seq)
        pts = [None] * N
        for j in range(N // 2):
            i0, i1 = 2 * j, 2 * j + 1
            pts[i0] = emit_exp(i0, stps.pop(i0))
            pts[i1] = emit_exp(i1, stps.pop(i1))
            if j >= 2:
                # lag-4: pv trails the exp pacer by two full cycles, so a
                # new (q,m) group's first pv (WAR on the single ot_ps buf)
                # never stalls on the previous group's 2.6us DVE
                # normalize chain
                emit_pv(seq[i0 - 4], i0, pts[i0 - 4])
                pts[i0 - 4] = None
                emit_pv(seq[i1 - 4], i1, pts[i1 - 4])
                pts[i1 - 4] = None
            drain(i1)
            if i1 + 2 < N:
                stps[i0 + 2] = st_mms(*seq[i0 + 2])
                stps[i1 + 2] = st_mms(*seq[i1 + 2])
        # collapse the lag at the end: the PE is about to idle, so the
        # write->read turnaround no longer costs anything
        for i in range(N - 4, N):
            emit_pv(seq[i], N + (i - (N - 4)), pts[i])
        drain(10 ** 9)

    nc.compile()
    return nc


def _get_built(dt_name="bfloat16"):
    if dt_name not in _BUILT:
        _BUILT[dt_name] = _build(dt_name)
    return _BUILT[dt_name]


def _prep_core_inputs(c, hidden_states, attention_mask, Wq, bq, Wk, bk, Wv, bv, Wo, bo,
                      np_dt):
    b, g = c // 4, c % 4
    hs = slice(g * HDC, (g + 1) * HDC)
    xtT = hidden_states[b].T.astype(np_dt)          # [H, S]
    # xt[c, p, ko, s'] = X^T[ko*128+p, c*512+s']
    xt = np.ascontiguousarray(
        xtT.reshape(KO, P, NQ, 512).transpose(2, 1, 0, 3))

    def wqkv(W):  # [H, HDC] -> [P, KO, HDC]
        return np.ascontiguousarray(
            W[:, hs].astype(np_dt).reshape(KO, P, HDC).transpose(1, 0, 2))

    mb = np.where(attention_mask[b] == 0, np.float32(-30000.0), np.float32(0.0))
    small = np.concatenate([
        np.ascontiguousarray(bq[hs].reshape(M2, P).T).astype(np.float32),
        np.ascontiguousarray(bk[hs].reshape(M2, P).T).astype(np.float32),
        np.tile(bv[hs].astype(np.float32), (P, 1)),
        np.ascontiguousarray(mb.astype(np.float32).reshape(ST_TILES, P).T),
    ], axis=1)
    return {
        "xt": xt,
        "wq": wqkv(Wq),
        "wk": wqkv(Wk),
        "wv": wqkv(Wv),
        "wo": np.ascontiguousarray(
            Wo[hs, :].astype(np_dt).reshape(M2, P, H).transpose(1, 0, 2)),
        "small": np.ascontiguousarray(small),
    }


def kernel(hidden_states, attention_mask, Wq, bq, Wk, bk, Wv, bv, Wo, bo,
           _trace=False, _trace_kwargs=None):
    from concourse.bass_utils import run_bass_kernel_spmd

    hidden_states = np.asarray(hidden_states, np.float32)
    attention_mask = np.asarray(attention_mask)
    Wq, bq = np.asarray(Wq, np.float32), np.asarray(bq, np.float32)
    Wk, bk = np.asarray(Wk, np.float32), np.asarray(bk, np.float32)
    Wv, bv = np.asarray(Wv, np.float32), np.asarray(bv, np.float32)
    Wo, bo = np.asarray(Wo, np.float32), np.asarray(bo, np.float32)

    nc = _get_built()
    np_dt = ml_dtypes.bfloat16
    in_maps = [
        _prep_core_inputs(c, hidden_states, attention_mask,
                          Wq, bq, Wk, bk, Wv, bv, Wo, bo, np_dt)
        for c in range(8)
    ]
    kwargs = {}
    if _trace:
        kwargs["trace"] = True
        if _trace_kwargs:
            kwargs.update(_trace_kwargs)
    res = run_bass_kernel_spmd(nc, in_maps, core_ids=list(range(8)), **kwargs)
    out = np.empty((B, S, H), np.float32)
    for b in range(B):
        acc = res.results[4 * b]["y"].astype(np.float32).copy()
        for c in range(4 * b + 1, 4 * b + 4):
            acc += res.results[c]["y"]
        out[b] = acc + bo[None, :]
    if _trace:
        return out, res
    return out



# revision 34
# speedup vs baseline: 1.0708x; 1.0083x over previous
"""Multi-head attention (B=2, S=2048, H=1024, 16 heads x 64) on 8 trn2 cores.

Sharding: core c handles batch b=c//4 and the 4 heads [4*(c%4) .. 4*(c%4)+3]
(tensor-parallel over the hd=256 column slice of Wq/Wk/Wv and the matching
row slice of Wo).  Each core computes a rank-256 partial of the output
projection for its batch; the host sums the 4 partials per batch and adds bo.

Device kernel (per core, bf16 matmuls with fp32 PSUM accumulate):
  QT[hd,s] = Wq_c^T X_b^T   (lhsT=Wq nat. layout, rhs=X^T prepped on host)
  KT[hd,s] similarly, stored zero-padded to K=128 per head slot (K=64
  matmuls stream ~1.5x slower per column on the PE), V[s,hd] as [ones|V_h].
  Per head pair (2 heads packed in the 128-partition dim):
    ST[k,q]  = [KT_h;0]^T QT_pair    (full K=128, zero rows null the other
                                      head's rows of the shared qt rhs)
    PT       = exp(SCALE*ST + maskbias[k])   (ScalarE, mask folded into bias)
    [d;OT]   = [ones64|V_h]^T PT     (M=128: rows 0:64 d replicated, 64:128 OT
                                      -> no separate denominator matmul)
    OT_norm  = OT * (1/d)            (reciprocal_approx_fast at base
                                      partition 0 + cross-partition DVE mults)
  Y_partial[s,H] = OT_norm^T Wo_c   (fp16 output tiles, host sums partials)

Scheduling: one flat software pipeline; ScalarE's exp paces the attention
stream while the PE runs PV two iterations behind it (avoids the psum
handoff + fresh-SBUF-read latency on pt).  Projection groups (QT/KT/V) and
output tiles drain from a deadline/credit-paced backlog between iterations
so the PE queue stays fed for the whole run.  PSUM pool creation order is
load-bearing (bank-group conflicts between the exp read stream and
concurrent matmul writes; see the pool comments).
"""
import sys

sys.path.insert(0, "/opt/trn_rl_repo")

import numpy as np
import ml_dtypes
from contextlib import ExitStack

B, S, H = 2, 2048, 1024
NH, HD = 16, 64
SCALE = 1.0 / float(np.sqrt(HD))
HPC = 4          # heads per core
HDC = HPC * HD   # 256 per-core head-dim slice
P = 128
KO = H // P      # 8 contraction tiles for the projections
ST_TILES = S // P    # 16
NQ = S // 512        # 4 q-chunks of 512
M2 = HDC // P        # 2 partition-tiles of the per-core head dim

_BUILT = {}


def _build(dt_name="bfloat16"):
    import concourse.bacc as bacc
    import concourse.mybir as mybir
    import concourse.tile as tile

    DT = getattr(mybir.dt, dt_name)
    F32 = mybir.dt.float32

    nc = bacc.Bacc("TRN2", target_bir_lowering=False, debug=False)

    # all inputs pre-rearranged on host so DMAs are per-partition contiguous
    xt_d = nc.dram_tensor("xt", [NQ, P, KO, 512], DT, kind="ExternalInput").ap()
    wq_d = nc.dram_tensor("wq", [P, KO, HDC], DT, kind="ExternalInput").ap()
    wk_d = nc.dram_tensor("wk", [P, KO, HDC], DT, kind="ExternalInput").ap()
    wv_d = nc.dram_tensor("wv", [P, KO, HDC], DT, kind="ExternalInput").ap()
    wo_d = nc.dram_tensor("wo", [P, M2, H], DT, kind="ExternalInput").ap()
    small_d = nc.dram_tensor("small", [P, M2 + M2 + HDC + ST_TILES], F32,
                             kind="ExternalInput").ap()
    F16 = mybir.dt.float16
    y_d = nc.dram_tensor("y", [S, H], F16, kind="ExternalOutput").ap()

    with tile.TileContext(nc) as tc, ExitStack() as ctx:
        consts = ctx.enter_context(tc.tile_pool(name="consts", bufs=1))
        y_pool = ctx.enter_context(tc.tile_pool(name="ysb", bufs=4))
        qkv = ctx.enter_context(tc.tile_pool(name="qkv", bufs=1))
        pt_pool = ctx.enter_context(tc.tile_pool(name="pt", bufs=9))
        sm_pool = ctx.enter_context(tc.tile_pool(name="sm", bufs=4))
        # PSUM bank-group placement matters: this order puts proj in {0,1},
        # the two ST buffers in {2,3}/{4,5} (so the exp read alternates
        # 4-bank groups) and the PV pair in {6,7} -- the measured conflict
        # tax is ~16us; other orders measured up to +40us worse.
        ps_proj = ctx.enter_context(tc.tile_pool(name="ps_proj", bufs=2, space="PSUM"))
        ps_st = ctx.enter_context(tc.tile_pool(name="ps_st", bufs=2, space="PSUM"))
        ps_ot = ctx.enter_context(tc.tile_pool(name="ps_ot", bufs=1, space="PSUM"))

        # ---- engine warmup during the input-DMA window ----
        # ~8 dummy matmuls trip the PE HAM clock-gate (3.4us busy window ->
        # 2.4GHz) and a dummy exp preloads the ScalarE Exp table, both before
        # any real work depends on them.
        ones_sb = consts.tile([P, 64], DT)
        nc.vector.memset(ones_sb[:], 1.0)
        warm_sb = consts.tile([P, 512], DT)
        nc.vector.memset(warm_sb[:], 1.0)
        warm_out = consts.tile([P, 64], DT)
        warm_ps = ps_proj.tile([P, 512], F32, tag="ps", name="warm_ps")
        for _ in range(18):
            nc.tensor.matmul(warm_ps[:], lhsT=warm_sb[:, 0:128], rhs=warm_sb[:],
                             start=True, stop=True)
        nc.scalar.activation(warm_out[:], ones_sb[:],
                             mybir.ActivationFunctionType.Exp,
                             bias=0.0, scale=1.0)

        # ---- input DMAs, strictly criticality-ordered ----
        # The first exp is gated on wk + xt chunk0 -> kt0/qt0 groups -> st0.
        # Those two transfers get the two fastest slots with nothing else
        # competing for HBM: scalar queue runs wk,wq,small,wo; sync queue
        # runs chunk0,wv,chunk1..3.  gpsimd stays empty so it never steals
        # bandwidth from the critical pair (measured: chunk0 landing at
        # ~21us with 3-way competition vs ~13us without).
        # wk and xt chunk0 split in halves: the kt0 group's first 4 matmuls
        # start as soon as the first halves land (~2us earlier than waiting
        # for the full transfers)
        wk_sb = consts.tile([P, KO, HDC], DT)
        nc.scalar.dma_start(wk_sb[:, 0:4], wk_d[:, 0:4])
        xt_sb = consts.tile([P, KO, S], DT)
        nc.sync.dma_start(xt_sb[:, 0:4, 0:512], xt_d[0][:, 0:4])
        nc.scalar.dma_start(wk_sb[:, 4:8], wk_d[:, 4:8])
        nc.sync.dma_start(xt_sb[:, 4:8, 0:512], xt_d[0][:, 4:8])
        wq_sb = consts.tile([P, KO, HDC], DT)
        nc.scalar.dma_start(wq_sb[:], wq_d)
        wv_sb = consts.tile([P, KO, HDC], DT)
        nc.sync.dma_start(wv_sb[:], wv_d)

        # small per-partition constants ride one packed DMA (dma_start issue
        # cost on the sequencers is ~0.6us each)
        small_sb = consts.tile([P, M2 + M2 + HDC + ST_TILES], F32)
        nc.scalar.dma_start(small_sb[:], small_d)
        bqt_sb = small_sb[:, 0:M2]
        bkt_sb = small_sb[:, M2:2 * M2]
        bvr_sb = small_sb[:, 2 * M2:2 * M2 + HDC]
        mb_sb = small_sb[:, 2 * M2 + HDC:]

        nc.sync.dma_start(xt_sb[:, :, 512:1024], xt_d[1])
        nc.sync.dma_start(xt_sb[:, :, 1024:1536], xt_d[2])
        nc.sync.dma_start(xt_sb[:, :, 1536:2048], xt_d[3])
        wo_sb = consts.tile([P, M2, H], DT)
        nc.scalar.dma_start(wo_sb[:], wo_d)

        qt_sb = qkv.tile([P, M2, S], DT)
        # KT natural layout [hd-pair 128, m, S]: rows 0:64 head 2m, rows
        # 64:128 head 2m+1.  The ST matmuls run as two CONCURRENT K=64
        # row-tiles (tile_position (0,0)/(64,0) auto-derived from the
        # base partitions) -- both heads' score tiles stream in ~512
        # cycles instead of 1024 with the old zero-padded K=128 form.
        kt_sb = qkv.tile([P, M2, S], DT)
        # per head h: cols 0:64 = ones, cols 64:128 = V_h (so the PV lhsT
        # [ones|V_h] computes the softmax denominator replicated across rows
        # 0:64 -- base partition 0, where reciprocal_approx_fast works --
        # and OT in rows 64:128, in the same matmul)
        v_sb = qkv.tile([P, ST_TILES, HPC, P], DT)
        nc.vector.memset(v_sb[:, :, :, 0:HD], 1.0)
        # one OT tile per q-chunk: separate tile objects keep the dependency
        # tracker from serializing q2's y tiles behind q3's normalize writes
        # (coarse per-tile write tracking created a false dep that defeated
        # the tail PE-filler)
        ot_sb = [qkv.tile([P, M2, 512], DT, name=f"ot_sb{q}")
                 for q in range(NQ)]

        # ---- projection group emitters ----
        # A spec describes one 8-matmul accumulation group; emitting two specs
        # interleaved lets each group's LDWEIGHTS prefetch under the other
        # group's matmul streaming.
        def _qk_ops(args):
            w_sb, b_sb, out_sb, m, q = args
            qs = slice(q * 512, (q + 1) * 512)
            ps = ps_proj.tile([P, 512], F32, tag="ps", name="ps_qk")

            def mm(ko, start, stop):
                nc.tensor.matmul(
                    ps[:],
                    lhsT=w_sb[:, ko, m * P:(m + 1) * P],
                    rhs=xt_sb[:, ko, qs],
                    start=start, stop=stop,
                )

            def finish():
                nc.vector.tensor_add(
                    out_sb[:, m, qs], ps[:],
                    b_sb[:, m:m + 1].to_broadcast((P, 512)),
                )

            return mm, finish

        def _v_ops(args):
            (st,) = args
            ps_full = ps_proj.tile([P, 512], F32, tag="ps", name="ps_v")
            ps = ps_full[:, :HDC]

            def mm(ko, start, stop):
                nc.tensor.matmul(
                    ps[:],
                    lhsT=xt_sb[:, ko, st * P:(st + 1) * P],
                    rhs=wv_sb[:, ko, :],
                    start=start, stop=stop,
                )

            def finish():
                for h in range(HPC):
                    hs = slice(h * HD, (h + 1) * HD)
                    nc.vector.tensor_add(v_sb[:, st, h, HD:P], ps[:, hs],
                                         bvr_sb[:, hs])

            return mm, finish

        def emit_group(kind, args):
            mm, finish = (_qk_ops if kind == "qk" else _v_ops)(args)
            for ko in range(KO):
                mm(ko, ko == 0, ko == KO - 1)
            finish()

        def emit_proj(specs):
            for kind, args in specs:
                emit_group(kind, args)

        # qk groups drained mid-stream run as two 4-matmul halves in
        # CONSECUTIVE drain pops (the half-group is ~1.1us of PE, which fits
        # the per-iteration slack beside the exp-paced ST/PV stream; a full
        # 2.2us group stalls the next ST and opens an exp gap).  halves of
        # one group are adjacent in the backlog (same deadline, consecutive
        # seq) so no other ps_proj user can slip between them.
        def emit_qk_half(args, half):
            w_sb, b_sb, out_sb, m, q, holder = args
            if half == 0:
                holder[0] = _qk_ops((w_sb, b_sb, out_sb, m, q))
                mm, _ = holder[0]
                for ko in range(4):
                    mm(ko, ko == 0, False)
            else:
                mm, finish = holder[0]
                for ko in range(4, KO):
                    mm(ko, False, ko == KO - 1)
                finish()
                holder[0] = None

        def emit_y_tile(st, n):
            ss = slice(st * P, (st + 1) * P)
            ns = slice(n * 512, (n + 1) * 512)
            yp = ps_proj.tile([P, 512], F32, tag="ps", name="yp")
            for m in range(M2):
                nc.tensor.matmul(
                    yp[:],
                    lhsT=ot_sb[st // 4][:, m, (st % 4) * P:(st % 4 + 1) * P],
                    rhs=wo_sb[:, m, ns],
                    start=(m == 0), stop=(m == M2 - 1),
                )
            y_sb = y_pool.tile([P, 512], F16, name="y_sb")
            # psum->sbuf copies ride the DVE during the exp stream (a ScalarE
            # copy steals ~0.7us from the exp pacer); only the final q-group
            # -- after the last exp -- alternates onto the idle ScalarE so
            # the tail copy chain runs on two engines.
            if st >= 12 and (st * 2 + n) % 2 == 1:
                nc.scalar.activation(y_sb[:], yp[:],
                                     mybir.ActivationFunctionType.Copy)
            else:
                nc.vector.tensor_copy(y_sb[:], yp[:])
            if (st * 2 + n) % 2 == 0:
                nc.sync.dma_start(y_d[ss, ns], y_sb[:])
            else:
                nc.gpsimd.dma_start(y_d[ss, ns], y_sb[:])

        # ---- backlog of work drained through the pipeline ----
        # items: (deadline_iter, seq, kind, args, pe_ns, release); kept
        # sorted by (deadline, seq).  release gates credit-pops: an item
        # never runs before iteration `release` (used to park PE filler for
        # the post-stream tail so HAM stays warm through the final
        # normalize).
        backlog = []
        _seq_no = [0]
        _COST = {"qkA": 1120, "qkB": 1120, "v": 920, "y": 490}

        def add(deadline, kind, *args, release=0):
            backlog.append((deadline, _seq_no[0], kind, args, _COST[kind],
                            release))
            _seq_no[0] += 1

        def add_qk(deadline, w_sb, b_sb, out_sb, m, q):
            holder = [None]
            add(deadline, "qkA", w_sb, b_sb, out_sb, m, q, holder)
            add(deadline + 1, "qkB", w_sb, b_sb, out_sb, m, q, holder)

        def run_item(kind, args):
            if kind == "y":
                emit_y_tile(*args)
            elif kind == "qkA":
                emit_qk_half(args, 0)
            elif kind == "qkB":
                emit_qk_half(args, 1)
            else:
                emit_group(kind, args)

        # discretionary drain pacing: PE has ~500ns/iter spare beside the
        # exp-paced ST/PV stream, but total backlog exceeds total spare, so
        # feed steadily to keep the PE queue non-empty for the whole
        # attention phase instead of exhausting the backlog early and
        # starving the PE late.
        _credit = [0.0]

        def drain(i):
            backlog.sort(key=lambda t: (t[0], t[1]))
            _credit[0] += 500.0
            while backlog:
                dl, _, kind, args, cost, rel = backlog[0]
                if rel > i:
                    break
                if dl <= i + 2 or _credit[0] >= cost:
                    backlog.pop(0)
                    run_item(kind, args)
                    _credit[0] = max(_credit[0] - cost, -2240.0)
                else:
                    break

        # attention group order: q-outer spreads Y work across the pipeline
        seq = [(q, m, kt) for q in range(NQ) for m in range(M2)
               for kt in range(ST_TILES)]
        giter = {}  # (q, m) -> start iter
        for i, (q, m, kt) in enumerate(seq):
            if kt == 0:
                giter[(q, m)] = i

        # prefix: just enough for the pipeline to start (v0/v1 are emitted
        # right after the first ST matmuls below, so the first exp isn't
        # queued behind them)

        # backlog deadlines
        for m in range(M2):
            for j in range(NQ):
                if (m, j) != (0, 0):
                    # KT chunk j needed by kt=4j of every group of this m
                    add_qk(giter[(0, m)] + 4 * j - 1, wk_sb, bkt_sb, kt_sb,
                           m, j)
                if (m, j) != (0, 0):
                    # -3: the finishing DVE bias-add must land a couple of
                    # iterations before the group boundary's first ST reads
                    # qt, else the in-order PE queue bubbles ~0.6us
                    add_qk(giter[(j, m)] - 3, wq_sb, bqt_sb, qt_sb, m, j)
        for st in range(2, ST_TILES):
            add(st, "v", st)

        # ---- flat attention pipeline ----
        def st_mms(q, m, kt):
            ks = slice(kt * P, (kt + 1) * P)
            qs = slice(q * 512, (q + 1) * 512)
            stp = ps_st.tile([P, 1024], F32, name="stp", tag="stp")
            # two concurrent K=64 row-tiles (auto tile_position (0,0) and
            # (64,0)): head A scores to cols 0:512, head B to 512:1024
            nc.tensor.matmul(
                stp[:, 0:512],
                lhsT=kt_sb[0:64, m, ks], rhs=qt_sb[0:64, m, qs],
                start=True, stop=True,
            )
            nc.tensor.matmul(
                stp[:, 512:1024],
                lhsT=kt_sb[64:128, m, ks], rhs=qt_sb[64:128, m, qs],
                start=True, stop=True,
            )
            return stp

        # PV runs two iterations BEHIND the exp pacer: at iter i the PE
        # consumes pt_{i-2}.  With lag 1 the PE (faster per-iter than
        # ScalarE once the backlog drains) catches up and stalls ~160ns per
        # iteration waiting on exp_{i-1} completing plus the fresh SBUF
        # write->read turnaround on pt; lag 2 gives a full exp period of
        # slack.
        _ot_ps = [None]

        def emit_pv(idx, i_emit, pt):
            q, m, kt = idx
            qs = slice(q * 512, (q + 1) * 512)
            hA, hB = 2 * m, 2 * m + 1
            if kt == 0:
                _ot_ps[0] = ps_ot.tile([P, 1024], F32, name="ot_ps")
            ot_ps = _ot_ps[0]
            # [d;OT] accumulation: lhsT [ones|V_h] -> rows 0:64 denominator
            # replicated, rows 64:128 OT_h (head A cols 0:512 of the pair
            # psum, head B cols 512:1024)
            nc.tensor.matmul(
                ot_ps[:, 0:512],
                lhsT=v_sb[:, kt, hA, :], rhs=pt[:, 0:512],
                start=(kt == 0), stop=(kt == ST_TILES - 1),
            )
            nc.tensor.matmul(
                ot_ps[:, 512:1024],
                lhsT=v_sb[:, kt, hB, :], rhs=pt[:, 512:1024],
                start=(kt == 0), stop=(kt == ST_TILES - 1),
            )
            if kt == ST_TILES - 1:
                rec = sm_pool.tile([P, 1024], F32, tag="rec", name="rec")
                nc.vector.reciprocal_approx_fast(rec[0:64, :], ot_ps[0:64, :])
                nc.vector.tensor_mul(ot_sb[q][0:64, m, :],
                                     ot_ps[64:128, 0:512],
                                     rec[0:64, 0:512])
                nc.vector.tensor_mul(ot_sb[q][64:128, m, :],
                                     ot_ps[64:128, 512:1024],
                                     rec[0:64, 512:1024])
                if m == M2 - 1:
                    idx2 = 0
                    for st in range(q * 4, q * 4 + 4):
                        for n in range(2):
                            if q == NQ - 2 and idx2 >= 4:
                                # park 4 of q2's y tiles as tail filler:
                                # they run right after the last exp, keeping
                                # the PE busy (HAM warm) through the final
                                # group's DVE normalize so q3's y matmuls
                                # run at 2.4GHz instead of re-throttled
                                add(124 + idx2, "y", st, n, release=128)
                            else:
                                add(i_emit + 2 + 2 * idx2, "y", st, n)
                            idx2 += 1

        def emit_exp(i, stp):
            kt = seq[i][2]
            pt = pt_pool.tile([P, 1024], DT, name="pt")
            nc.scalar.activation(
                pt[:], stp[:],
                mybir.ActivationFunctionType.Exp,
                bias=mb_sb[:, kt:kt + 1],
                scale=SCALE,
            )
            return pt

        emit_proj([("qk", (wk_sb, bkt_sb, kt_sb, 0, 0)),   # KT m0 chunk 0
                   ("qk", (wq_sb, bqt_sb, qt_sb, 0, 0))])  # QT m0 q0
        stps = {0: st_mms(*seq[0])}
        emit_proj([("v", (0,))])
        stps[1] = st_mms(*seq[1])
        emit_proj([("v", (1,))])

        # PAIR-CYCLED pipeline: cycle j covers iterations (2j, 2j+1).
        # Per-cycle PE order: [pv(2j-2) pv(2j-1) drain] (all 128-contraction
        # mode) then [st(2j+2) st(2j+3)] (one 64-row-mode island) -- one
        # tiling-mode flip per cycle instead of two, and by the time the PE
        # works through the ~1.8us pv+drain block, exp(2j) has finished
        # reading the psum buf st(2j+2) wants to overwrite, so the in-order
        # PE queue doesn't stall on the WAR dependency.  PV keeps a lag of
        # one full cycle (2 iterations) behind the exp pacer.
        N = len(seq)
        pts = [None] * N
        for j in range(N // 2):
            i0, i1 = 2 * j, 2 * j + 1
            pts[i0] = emit_exp(i0, stps.pop(i0))
            pts[i1] = emit_exp(i1, stps.pop(i1))
            if j >= 3:
                # lag-6: pv trails the exp pacer by three full cycles, so a
                # new (q,m) group's first pv (WAR on the single ot_ps buf)
                # never stalls on the previous group's ~2.7us DVE
                # normalize chain (rec + 2 muls, serial on DVE)
                emit_pv(seq[i0 - 6], i0, pts[i0 - 6])
                pts[i0 - 6] = None
                emit_pv(seq[i1 - 6], i1, pts[i1 - 6])
                pts[i1 - 6] = None
            drain(i1)
            if i1 + 2 < N:
                stps[i0 + 2] = st_mms(*seq[i0 + 2])
                stps[i1 + 2] = st_mms(*seq[i1 + 2])
        # collapse the lag at the end: the PE is about to idle, so the
        # write->read turnaround no longer costs anything
        for i in range(N - 6, N):
            emit_pv(seq[i], N + (i - (N - 6)), pts[i])
        drain(10 ** 9)

    nc.compile()
    return nc


def _get_built(dt_name="bfloat16"):
    if dt_name not in _BUILT:
        _BUILT[dt_name] = _build(dt_name)
    return _BUILT[dt_name]


def _prep_core_inputs(c, hidden_states, attention_mask, Wq, bq, Wk, bk, Wv, bv, Wo, bo,
                      np_dt):
    b, g = c // 4, c % 4
    hs = slice(g * HDC, (g + 1) * HDC)
    xtT = hidden_states[b].T.astype(np_dt)          # [H, S]
    # xt[c, p, ko, s'] = X^T[ko*128+p, c*512+s']
    xt = np.ascontiguousarray(
        xtT.reshape(KO, P, NQ, 512).transpose(2, 1, 0, 3))

    def wqkv(W):  # [H, HDC] -> [P, KO, HDC]
        return np.ascontiguousarray(
            W[:, hs].astype(np_dt).reshape(KO, P, HDC).transpose(1, 0, 2))

    mb = np.where(attention_mask[b] == 0, np.float32(-30000.0), np.float32(0.0))
    small = np.concatenate([
        np.ascontiguousarray(bq[hs].reshape(M2, P).T).astype(np.float32),
        np.ascontiguousarray(bk[hs].reshape(M2, P).T).astype(np.float32),
        np.tile(bv[hs].astype(np.float32), (P, 1)),
        np.ascontiguousarray(mb.astype(np.float32).reshape(ST_TILES, P).T),
    ], axis=1)
    return {
        "xt": xt,
        "wq": wqkv(Wq),
        "wk": wqkv(Wk),
        "wv": wqkv(Wv),
        "wo": np.ascontiguousarray(
            Wo[hs, :].astype(np_dt).reshape(M2, P, H).transpose(1, 0, 2)),
        "small": np.ascontiguousarray(small),
    }


def kernel(hidden_states, attention_mask, Wq, bq, Wk, bk, Wv, bv, Wo, bo,
           _trace=False, _trace_kwargs=None):
    from concourse.bass_utils import run_bass_kernel_spmd

    hidden_states = np.asarray(hidden_states, np.float32)
    attention_mask = np.asarray(attention_mask)
    Wq, bq = np.asarray(Wq, np.float32), np.asarray(bq, np.float32)
    Wk, bk = np.asarray(Wk, np.float32), np.asarray(bk, np.float32)
    Wv, bv = np.asarray(Wv, np.float32), np.asarray(bv, np.float32)
    Wo, bo = np.asarray(Wo, np.float32), np.asarray(bo, np.float32)

    nc = _get_built()
    np_dt = ml_dtypes.bfloat16
    in_maps = [
        _prep_core_inputs(c, hidden_states, attention_mask,
                          Wq, bq, Wk, bk, Wv, bv, Wo, bo, np_dt)
        for c in range(8)
    ]
    kwargs = {}
    if _trace:
        kwargs["trace"] = True
        if _trace_kwargs:
            kwargs.update(_trace_kwargs)
    res = run_bass_kernel_spmd(nc, in_maps, core_ids=list(range(8)), **kwargs)
    out = np.empty((B, S, H), np.float32)
    for b in range(B):
        acc = res.results[4 * b]["y"].astype(np.float32).copy()
        for c in range(4 * b + 1, 4 * b + 4):
            acc += res.results[c]["y"]
        out[b] = acc + bo[None, :]
    if _trace:
        return out, res
    return out

